# revision 56
# baseline (speedup 1.0000x reference)
"""BiLSTM-CRF Trainium2 kernel (Bass/Tile), data-parallel over batch on 8
NeuronCores. Self-contained: host prep + device emission + SPMD runner.

Pipeline per core (16 sequences, T=512):
  embedding gather (indirect DMA, fp16) -> PE transpose -> Wx matmuls (fp16),
  software-pipelined with the serial BiLSTM recurrence (gates on partitions,
  fp16 weights/hist, fp32 cell state) -> emission scores matmul -> blocked
  Viterbi forward scan + blocked backtrace (max-plus / one-hot map composition
  in 32 chunks of 16 steps, vectorized across 128 partitions).
"""
import sys
import types
import numpy as np

import concourse.bass as bass
import concourse.mybir as mybir
from concourse import tile
from concourse.vector_clock import ScopedClock
import bass_rust
from contextlib import ExitStack

F16 = mybir.dt.float16
F32 = mybir.dt.float32
I32 = mybir.dt.int32
AF = mybir.ActivationFunctionType
AX = mybir.AxisListType.X
OP = mybir.AluOpType

B_FULL, T, V, D = 128, 512, 8000, 256
NB = 16          # sequences per core
NCORES = 8


# ---------------------------------------------------------------------------
# Harness workarounds: walrus in this environment accepts only ONE sync-wait
# per instruction; split extras onto NoOps (BIR json pass) and chunk the Tile
# exit drain. Also register the NTFF profile hook shim so BASS_TRACE=1 works.
# ---------------------------------------------------------------------------
import json as _json

_SW_CTR = [0]


def _split_sync_waits(bir_json: bytes) -> bytes:
    d = _json.loads(bir_json)
    changed = False
    for fn in d.get("functions", []):
        for blk in fn.get("blocks", []):
            new_insts = []
            for inst in blk.get("instructions", []):
                si = inst.get("sync_info")
                waits = (si or {}).get("on_wait") or []
                if len(waits) > 1:
                    changed = True
                    for w in waits[:-1]:
                        _SW_CTR[0] += 1
                        nop = {
                            "engine": inst["engine"],
                            "ins": [],
                            "outs": [],
                            "name": f"I-swsplit-{_SW_CTR[0]}",
                            "opcode": "NoOp",
                            "sync_info": {"on_update": [], "on_wait": [w]},
                        }
                        if "debug" in inst:
                            nop["debug"] = inst["debug"]
                        new_insts.append(nop)
                    si["on_wait"] = [waits[-1]]
                new_insts.append(inst)
            blk["instructions"] = new_insts
    return _json.dumps(d).encode() if changed else bir_json


def _patched_drain_and_barrier(self, tick_clock, wait_clock):
    drain_inst = self.nc.sync.drain()
    wait_clock.add_sem_waits(
        drain_inst.ins, ScopedClock({None: tick_clock.global_clock})
    )
    si = drain_inst.ins.sync_info
    if si is not None and si.on_wait is not None and len(si.on_wait) > 1:
        waits = list(si.on_wait)
        drain_inst.ins.sync_info = bass_rust.SyncInfo(
            on_wait=waits[:1], on_update=list(si.on_update or [])
        )
        for i in range(1, len(waits)):
            nop = self.nc.sync.nop()
            nop.ins.sync_info = bass_rust.SyncInfo(on_wait=[waits[i]], on_update=[])
    self.nc.all_engine_barrier()
    assert self.sems is not None
    popped = self.nc._tile_sem_poison_stack.pop()
    assert popped is self._sem_poison
    self.nc.clear_and_free_semaphores(list(self.sems.allocated().values()))
    self.nc.all_engine_barrier()


_PATCHED = [False]


def _apply_patches():
    if _PATCHED[0]:
        return
    _PATCHED[0] = True
    tile.TileContext._drain_and_barrier = _patched_drain_and_barrier
    import concourse.bass_utils as _bu
    import concourse.bass2jax as _b2j

    _orig_compile = _bu.compile_bir_kernel

    def _wrapped(bir_json, tmpdir, neff_name="file.neff"):
        return _orig_compile(_split_sync_waits(bir_json), tmpdir, neff_name)

    _wrapped._swsplit_wrapped = True
    _bu.compile_bir_kernel = _wrapped
    _b2j.compile_bir_kernel = _wrapped

    if "antenv.axon_hooks" not in sys.modules:
        try:
            import trn_agent_boot.trn_boot as _tb
            _hook = _tb._ntff_profile_via_ctypes("/opt/axon/libaxon_pjrt.so")
        except Exception:
            _hook = None
        m = types.ModuleType("antenv.axon_hooks")
        m.get_axon_ntff_profile_hook = lambda: _hook
        m.set_axon_ntff_profile_hook = lambda h: None
        sys.modules["antenv.axon_hooks"] = m





def A(t, off, dims, p0=0):
    # t: pool tile AP [[rowsize, P], [1, rowsize]]. dims[0] is the partition
    # pair whose step is replaced by the tile's canonical per-partition row
    # size; off is the within-partition element offset.
    rs = t.ap[0][0]
    d = [list(x) for x in dims]
    d[0] = [rs, d[0][1]]
    return bass.AP(t.tensor, t.offset + p0 * rs + off, d)


def AD(handle, off, dims):
    return bass.AP(handle, off, [list(d) for d in dims])


def emit_crf(nc, tc, dr, pool):
    """dr: dict of DRAM handles. pool: sbuf tile pool to allocate from."""
    v = nc.vector

    # ---- V0: build T matrices ------------------------------------------
    scT = pool.tile([128, 256], F32)   # (chpos, s, c)
    # scores_dram is tok-major [8192, 4]: addr = (t*16+b)*4 + c
    for g in range(8):
        nc.sync.dma_start(
            A(scT, 0, [[1, 16], [4, 64], [1, 4]], p0=g * 16),
            AD(dr["scores"], g * 4096, [[4, 16], [64, 64], [1, 4]]),
        )
    transb_sb = pool.tile([128, 16], F32)
    nc.sync.dma_start(transb_sb[:], dr["transb16"][None, :].to_broadcast((128, 16)))
    imp_sb = pool.tile([128, 16], F32)
    nc.sync.dma_start(imp_sb[:], dr["impflat"][None, :].to_broadcast((128, 16)))
    vmask_sb = pool.tile([128, 1024], I32)
    nc.sync.dma_start(vmask_sb[:], dr["vmask128"][:])
    fromBp_sb = pool.tile([128, 4], F32)
    nc.sync.dma_start(fromBp_sb[:], dr["fromBp4"][None, :].to_broadcast((128, 4)))

    Traw = pool.tile([128, 1024], F32)  # (chpos, s, p, c)
    v.tensor_add(
        out=A(Traw, 0, [[1, 128], [256, 4], [16, 16], [4, 4], [1, 4]]),
        in0=A(scT, 0, [[1, 128], [64, 4], [4, 16], [0, 4], [1, 4]]),
        in1=A(transb_sb, 0, [[1, 128], [0, 4], [0, 16], [4, 4], [1, 4]]),
    )
    T128 = pool.tile([128, 1024], F32)
    v.select(
        out=A(T128, 0, [[1, 128], [256, 4], [16, 16], [4, 4], [1, 4]]),
        mask=A(vmask_sb, 0, [[1, 128], [256, 4], [16, 16], [4, 4], [1, 4]]),
        on_true=A(Traw, 0, [[1, 128], [256, 4], [16, 16], [4, 4], [1, 4]]),
        on_false=A(imp_sb, 0, [[1, 128], [0, 4], [0, 16], [4, 4], [1, 4]]),
    )
    # step 0 (partitions 0:16, chpos=0, s=0): T = e0 + fromBp (rows equal)
    v.tensor_add(
        out=A(T128, 0, [[1, 16], [4, 4], [1, 4]]),
        in0=A(scT, 0, [[1, 16], [0, 4], [1, 4]]),
        in1=A(fromBp_sb, 0, [[1, 16], [0, 4], [1, 4]]),
    )

    # ---- V1: chunk max-plus products -----------------------------------
    Ma = pool.tile([128, 64], F32)   # (chpos, i, k/j)
    Mb = pool.tile([128, 64], F32)
    tmp256 = pool.tile([128, 1024], F32)
    v.tensor_copy(
        A(Ma, 0, [[1, 128], [16, 4], [4, 4], [1, 4]]),
        A(T128, 0, [[1, 128], [256, 4], [4, 4], [1, 4]]),
    )
    cur, nxt = Ma, Mb
    for s in range(1, 16):
        v.tensor_add(
            out=A(tmp256, 0, [[1, 128], [64, 4], [16, 4], [4, 4], [1, 4]]),
            in0=A(cur, 0, [[1, 128], [16, 4], [4, 4], [1, 4], [0, 4]]),
            in1=A(T128, s * 16, [[1, 128], [256, 4], [0, 4], [4, 4], [1, 4]]),
        )
        v.tensor_reduce(
            out=A(nxt, 0, [[1, 128], [16, 4], [4, 4], [1, 4]]),
            in_=A(tmp256, 0, [[1, 128], [64, 4], [16, 4], [1, 4], [4, 4]]),
            axis=AX, op=OP.max,
        )
        cur, nxt = nxt, cur
    nc.sync.dma_start(
        AD(dr["mdram"], 0, [[64, 128], [1, 64]]),
        A(cur, 0, [[1, 128], [1, 64]]),
    )

    # ---- V2: serial chunk scan (16 partitions) -------------------------
    M16 = pool.tile([16, 512], F32)
    for g in range(8):
        nc.sync.dma_start(
            A(M16, g * 64, [[1, 16], [1, 64]]),
            AD(dr["mdram"], g * 1024, [[64, 16], [1, 64]]),
        )
    Ball = pool.tile([16, 132], F32)
    v.memset(Ball[:], 0.0)
    t16 = pool.tile([16, 16], F32)
    for c in range(32):
        v.tensor_add(
            out=A(t16, 0, [[1, 16], [4, 4], [1, 4]]),
            in0=A(Ball, c * 4, [[1, 16], [1, 4], [0, 4]]),
            in1=A(M16, c * 16, [[1, 16], [4, 4], [1, 4]]),
        )
        v.tensor_reduce(
            out=A(Ball, (c + 1) * 4, [[1, 16], [1, 4]]),
            in_=A(t16, 0, [[1, 16], [1, 4], [4, 4]]),
            axis=AX, op=OP.max,
        )
    # last label one-hot
    toEOS_sb = pool.tile([16, 4], F32)
    nc.sync.dma_start(toEOS_sb[:], dr["toEOS4"][None, :].to_broadcast((16, 4)))
    c3lab_sb = pool.tile([16, 4], F32)
    nc.sync.dma_start(c3lab_sb[:], dr["c3lab4"][None, :].to_broadcast((16, 4)))
    wiota16 = pool.tile([16, 4], F32)
    nc.sync.dma_start(wiota16[:], dr["wiota4"][None, :].to_broadcast((16, 4)))
    fin = pool.tile([16, 4], F32)
    v.tensor_add(out=fin[:], in0=A(Ball, 128, [[1, 16], [1, 4]]), in1=toEOS_sb[:])
    lmax = pool.tile([16, 1], F32)
    v.tensor_reduce(out=lmax[:], in_=fin[:], axis=AX, op=OP.max)
    loh = pool.tile([16, 4], F32)
    v.tensor_tensor(out=loh[:], in0=fin[:],
                    in1=A(lmax, 0, [[1, 16], [0, 4]]), op=OP.is_equal)
    lohm = pool.tile([16, 4], F32)
    v.tensor_mul(out=lohm[:], in0=loh[:], in1=c3lab_sb[:])
    lenc = pool.tile([16, 1], F32)
    v.tensor_reduce(out=lenc[:], in_=lohm[:], axis=AX, op=OP.max)
    llval = pool.tile([16, 1], F32)
    v.tensor_scalar(out=llval[:], in0=lenc[:], scalar1=-1.0, scalar2=3.0,
                    op0=OP.mult, op1=OP.add)
    lloh = pool.tile([16, 4], F32)
    v.tensor_tensor(out=lloh[:], in0=wiota16[:],
                    in1=A(llval, 0, [[1, 16], [0, 4]]), op=OP.is_equal)
    nc.sync.dma_start(AD(dr["lldram"], 0, [[4, 16], [1, 4]]), lloh[:])
    nc.sync.dma_start(AD(dr["edram"], 0, [[132, 16], [1, 132]]), Ball[:])

    # ---- V3: replay -> backtrace tables --------------------------------
    c3p_sb = pool.tile([128, 16], F32)
    nc.sync.dma_start(c3p_sb[:], dr["c3p16"][None, :].to_broadcast((128, 16)))
    bestA = pool.tile([128, 16], F32)
    bestB = pool.tile([128, 16], F32)
    nc.sync.dma_start(
        bestA[:], AD(dr["edram"], 0, [[16, 8], [132, 16], [4, 4], [1, 4]])
    )
    BT = pool.tile([128, 256], F32)     # (chpos, s, c)
    smat = pool.tile([128, 64], F32)
    oh64 = pool.tile([128, 64], F32)
    enc128 = pool.tile([128, 16], F32)
    bcur, bnxt = bestA, bestB
    for s in range(16):
        v.tensor_add(
            out=A(smat, 0, [[1, 128], [16, 4], [4, 4], [1, 4]]),
            in0=A(bcur, 0, [[1, 128], [4, 4], [1, 4], [0, 4]]),
            in1=A(T128, s * 16, [[1, 128], [256, 4], [4, 4], [1, 4]]),
        )
        v.tensor_reduce(
            out=A(bnxt, 0, [[1, 128], [4, 4], [1, 4]]),
            in_=A(smat, 0, [[1, 128], [16, 4], [1, 4], [4, 4]]),
            axis=AX, op=OP.max,
        )
        v.tensor_tensor(
            out=A(oh64, 0, [[1, 128], [16, 4], [4, 4], [1, 4]]),
            in0=A(smat, 0, [[1, 128], [16, 4], [4, 4], [1, 4]]),
            in1=A(bnxt, 0, [[1, 128], [4, 4], [0, 4], [1, 4]]),
            op=OP.is_equal,
        )
        v.tensor_mul(
            out=A(oh64, 0, [[1, 128], [16, 4], [4, 4], [1, 4]]),
            in0=A(oh64, 0, [[1, 128], [16, 4], [4, 4], [1, 4]]),
            in1=A(c3p_sb, 0, [[1, 128], [0, 4], [4, 4], [1, 4]]),
        )
        v.tensor_reduce(
            out=A(enc128, 0, [[1, 128], [4, 4], [1, 4]]),
            in_=A(oh64, 0, [[1, 128], [16, 4], [1, 4], [4, 4]]),
            axis=AX, op=OP.max,
        )
        v.tensor_scalar(
            out=A(BT, s * 4, [[1, 128], [64, 4], [1, 4]]),
            in0=A(enc128, 0, [[1, 128], [4, 4], [1, 4]]),
            scalar1=-1.0, scalar2=3.0, op0=OP.mult, op1=OP.add,
        )
        bcur, bnxt = bnxt, bcur
    nc.sync.dma_start(
        AD(dr["btdram"], 0, [[256, 128], [1, 256]]),
        A(BT, 0, [[1, 128], [1, 256]]),
    )

    # ---- VA: backtrace map tables + chunk compositions -----------------
    BTS = pool.tile([128, 256], F32)
    # top group's last slot is never used; zero-fill before partial overwrite
    v.memset(A(BTS, 252, [[1, 128], [1, 4]]), 0.0)
    nc.sync.dma_start(
        A(BTS, 0, [[1, 128], [1, 252]]),
        AD(dr["btdram"], 4, [[256, 128], [1, 252]]),
    )
    # last slot of each partition: first bt entry of the next chunk group
    nc.sync.dma_start(
        A(BTS, 252, [[1, 112], [1, 4]]),
        AD(dr["btdram"], 16 * 256, [[256, 112], [1, 4]]),
    )
    meq_sb = pool.tile([128, 64], I32)
    mlt_sb = pool.tile([128, 64], I32)
    nc.sync.dma_start(meq_sb[:], dr["meq128"][:])
    nc.sync.dma_start(mlt_sb[:], dr["mlt128"][:])
    lloh128 = pool.tile([128, 4], F32)
    nc.sync.dma_start(lloh128[:], AD(dr["lldram"], 0, [[0, 8], [4, 16], [1, 4]]))
    i4_sb = pool.tile([128, 16], F32)
    nc.sync.dma_start(i4_sb[:], dr["i4flat"][None, :].to_broadcast((128, 16)))
    wiota128 = pool.tile([128, 4], F32)
    nc.sync.dma_start(wiota128[:], dr["wiota4"][None, :].to_broadcast((128, 4)))

    Fall = pool.tile([128, 1024], F32)  # (chpos, s, u, w)
    tmpA = pool.tile([128, 64], F32)
    for s in range(16):
        # oh(u,w) = bt_{t+1}[u] == w
        v.tensor_tensor(
            out=A(tmpA, 0, [[1, 128], [16, 4], [4, 4], [1, 4]]),
            in0=A(BTS, s * 4, [[1, 128], [64, 4], [1, 4], [0, 4]]),
            in1=A(wiota128, 0, [[1, 128], [0, 4], [0, 4], [1, 4]]),
            op=OP.is_equal,
        )
        # tmp2 = meq ? lloh : I4  ; F = mlt ? oh : tmp2  (write into Fall)
        v.select(
            out=A(Fall, s * 16, [[1, 128], [256, 4], [4, 4], [1, 4]]),
            mask=A(meq_sb, s, [[1, 128], [16, 4], [0, 4], [0, 4]]),
            on_true=A(lloh128, 0, [[1, 128], [0, 4], [0, 4], [1, 4]]),
            on_false=A(i4_sb, 0, [[1, 128], [0, 4], [4, 4], [1, 4]]),
        )
        v.select(
            out=A(Fall, s * 16, [[1, 128], [256, 4], [4, 4], [1, 4]]),
            mask=A(mlt_sb, s, [[1, 128], [16, 4], [0, 4], [0, 4]]),
            on_true=A(tmpA, 0, [[1, 128], [16, 4], [4, 4], [1, 4]]),
            on_false=A(Fall, s * 16, [[1, 128], [256, 4], [4, 4], [1, 4]]),
        )
    Ga = pool.tile([128, 64], F32)
    Gb = pool.tile([128, 64], F32)
    v.tensor_copy(
        A(Ga, 0, [[1, 128], [16, 4], [4, 4], [1, 4]]),
        A(Fall, 15 * 16, [[1, 128], [256, 4], [4, 4], [1, 4]]),
    )
    gcur, gnxt = Ga, Gb
    for s in range(14, -1, -1):
        v.tensor_mul(
            out=A(tmp256, 0, [[1, 128], [64, 4], [16, 4], [4, 4], [1, 4]]),
            in0=A(gcur, 0, [[1, 128], [16, 4], [4, 4], [1, 4], [0, 4]]),
            in1=A(Fall, s * 16, [[1, 128], [256, 4], [0, 4], [4, 4], [1, 4]]),
        )
        v.tensor_reduce(
            out=A(gnxt, 0, [[1, 128], [16, 4], [4, 4], [1, 4]]),
            in_=A(tmp256, 0, [[1, 128], [64, 4], [16, 4], [1, 4], [4, 4]]),
            axis=AX, op=OP.max,
        )
        gcur, gnxt = gnxt, gcur
    nc.sync.dma_start(
        AD(dr["gdram"], 0, [[64, 128], [1, 64]]),
        A(gcur, 0, [[1, 128], [1, 64]]),
    )

    # ---- VB: serial reverse chunk scan (16 partitions) -----------------
    Gall16 = pool.tile([16, 512], F32)
    for g in range(8):
        nc.sync.dma_start(
            A(Gall16, g * 64, [[1, 16], [1, 64]]),
            AD(dr["gdram"], g * 1024, [[64, 16], [1, 64]]),
        )
    EB = pool.tile([16, 132], F32)
    nc.sync.dma_start(
        A(EB, 128, [[1, 16], [1, 4]]), dr["e0oh4"][None, :].to_broadcast((16, 4))
    )
    tb16 = pool.tile([16, 16], F32)
    for c in range(31, -1, -1):
        v.tensor_mul(
            out=tb16[:],
            in0=A(EB, (c + 1) * 4, [[1, 16], [1, 4], [0, 4]]),
            in1=A(Gall16, c * 16, [[1, 16], [4, 4], [1, 4]]),
        )
        v.tensor_reduce(
            out=A(EB, c * 4, [[1, 16], [1, 4]]),
            in_=A(tb16, 0, [[1, 16], [1, 4], [4, 4]]),
            axis=AX, op=OP.max,
        )
    nc.sync.dma_start(AD(dr["ebdram"], 0, [[132, 16], [1, 132]]), EB[:])

    # ---- VC: labels -----------------------------------------------------
    cohE = pool.tile([128, 16], F32)
    nc.sync.dma_start(
        cohE[:], AD(dr["ebdram"], 4, [[16, 8], [132, 16], [4, 4], [1, 4]])
    )
    LABOH = pool.tile([128, 256], F32)  # (chpos, s, w)
    tmpc = pool.tile([128, 64], F32)
    for s in range(15, -1, -1):
        if s == 15:
            in0 = A(cohE, 0, [[1, 128], [4, 4], [1, 4], [0, 4]])
        else:
            in0 = A(LABOH, (s + 1) * 4, [[1, 128], [64, 4], [1, 4], [0, 4]])
        v.tensor_mul(
            out=A(tmpc, 0, [[1, 128], [16, 4], [4, 4], [1, 4]]),
            in0=in0,
            in1=A(Fall, s * 16, [[1, 128], [256, 4], [4, 4], [1, 4]]),
        )
        v.tensor_reduce(
            out=A(LABOH, s * 4, [[1, 128], [64, 4], [1, 4]]),
            in_=A(tmpc, 0, [[1, 128], [16, 4], [1, 4], [4, 4]]),
            axis=AX, op=OP.max,
        )
    omask_sb = pool.tile([128, 64], F32)
    nc.sync.dma_start(omask_sb[:], dr["outmask128"][:])
    labv = pool.tile([128, 64], F32)
    tmpl = pool.tile([128, 256], F32)
    v.tensor_mul(
        out=A(tmpl, 0, [[1, 128], [64, 4], [4, 16], [1, 4]]),
        in0=A(LABOH, 0, [[1, 128], [64, 4], [4, 16], [1, 4]]),
        in1=A(wiota128, 0, [[1, 128], [0, 4], [0, 16], [1, 4]]),
    )
    v.tensor_reduce(
        out=A(labv, 0, [[1, 128], [16, 4], [1, 16]]),
        in_=A(tmpl, 0, [[1, 128], [64, 4], [4, 16], [1, 4]]),
        axis=AX, op=OP.add,
    )
    v.tensor_mul(out=labv[:], in0=labv[:], in1=omask_sb[:])
    labi = pool.tile([128, 64], I32)
    v.tensor_copy(labi[:], labv[:])
    for cp in range(4):
        nc.sync.dma_start(
            AD(dr["labels"], 16 * cp, [[64, 8], [512, 16], [1, 16]]),
            A(labi, cp * 16, [[1, 128], [1, 16]]),
        )


def host_crf_consts(lens, trans, fromB, toEOS, b_lab):
    """All host-side constant arrays, keyed to match dram handle names."""
    import numpy as np
    T, B, L = 512, 16, 4
    NEG = -1e9
    chgrp = np.arange(8)
    out = {}
    out["transb16"] = (trans + b_lab[None, :]).astype(np.float32).reshape(16)
    imp = np.full((L, L), NEG, np.float32)
    np.fill_diagonal(imp, 0.0)
    out["impflat"] = imp.reshape(16)
    out["fromBp4"] = (fromB + b_lab).astype(np.float32)
    out["toEOS4"] = toEOS.astype(np.float32)
    out["c3lab4"] = (3.0 - np.arange(4)).astype(np.float32)
    out["wiota4"] = np.arange(4).astype(np.float32)
    out["c3p16"] = np.repeat(3.0 - np.arange(4), 4).astype(np.float32)
    out["i4flat"] = np.eye(4, dtype=np.float32).reshape(16)
    out["e0oh4"] = np.array([1, 0, 0, 0], np.float32)
    # t value at (P, chpos, s):  P = chgrp*16 + b ; t = 16*(4*chgrp+chpos)+s
    P_chgrp = np.arange(128) // 16
    P_b = np.arange(128) % 16
    chpos = np.arange(4)
    s = np.arange(16)
    tt = 16 * (4 * P_chgrp[:, None, None] + chpos[None, :, None]) + s[None, None, :]
    lb = lens[P_b][:, None, None]
    vm = (tt < lb)
    out["vmask128"] = np.repeat(
        vm.reshape(128, 64)[:, :, None], 16, axis=2
    ).reshape(128, 1024).astype(np.int32)
    out["meq128"] = (tt == lb - 1).reshape(128, 64).astype(np.int32)
    out["mlt128"] = (tt < lb - 1).reshape(128, 64).astype(np.int32)
    out["outmask128"] = (tt < lb).reshape(128, 64).astype(np.float32)
    return out


CRF_DRAM_SPECS = [
    ("transb16", [16], F32), ("impflat", [16], F32), ("fromBp4", [4], F32),
    ("toEOS4", [4], F32), ("c3lab4", [4], F32), ("wiota4", [4], F32),
    ("c3p16", [16], F32), ("i4flat", [16], F32), ("e0oh4", [4], F32),
    ("vmask128", [128, 1024], I32), ("meq128", [128, 64], I32),
    ("mlt128", [128, 64], I32), ("outmask128", [128, 64], F32),
]
CRF_SCRATCH_SPECS = [
    ("mdram", [8192], F32), ("edram", [2112], F32), ("btdram", [32832], F32),
    ("gdram", [8192], F32), ("lldram", [64], F32), ("ebdram", [2112], F32),
]






class LstmEmitter:
    def __init__(self, nc, tc, dr, T, pools):
        self.nc, self.tc, self.dr, self.T = nc, tc, dr, T
        self.NBLK = T // 64
        p = pools
        hist_all = p["hist"].tile([128, 2 * (T + 1) * 16], F16, name="hist_all")
        self.hist_all = hist_all
        self.hb_off = (T + 1) * 16
        self.hist = {
            "f": hist_all[:, 0:(T + 1) * 16],
            "b": hist_all[:, (T + 1) * 16:2 * (T + 1) * 16],
        }
        nc.vector.memset(self.hist["f"][:, 0:16], 0.0)
        nc.vector.memset(self.hist["b"][:, T * 16:(T + 1) * 16], 0.0)
        self.cboth = p["state"].tile([128, 32], F32, name="cboth")
        nc.vector.memset(self.cboth[:], 0.0)
        # weights
        self.whhT = {}
        self.wihT = {}
        self.biasT = {}
        self.wlabT = {}
        for d in ("f", "b"):
            w = p["wts"].tile([128, 512], F16, name=f"whh_{d}")
            nc.sync.dma_start(w[:], dr[f"whhT_{d}"][:])
            self.whhT[d] = w
            hs = []
            for h in range(2):
                wh = p["wts"].tile([128, 512], F16, name=f"wih_{d}{h}")
                nc.sync.dma_start(wh[:], dr[f"wihT_{d}{h}"][:])
                hs.append(wh)
            self.wihT[d] = hs
            bt = p["wts"].tile([128, 4], F32, name=f"bias_{d}")
            nc.sync.dma_start(bt[:], dr[f"biasT_{d}"][:])
            self.biasT[d] = bt
            wl = p["wts"].tile([128, 4], F16, name=f"wlab_{d}")
            nc.sync.dma_start(wl[:], dr[f"wlabT_{d}"][:])
            self.wlabT[d] = wl
        self.ident = p["wts"].tile([128, 128], F16)
        nc.sync.dma_start(self.ident[:], dr["ident"][:])
        self.ident32 = p["wts"].tile([128, 16], F32, name="ident32")
        nc.sync.dma_start(self.ident32[:], dr["ident32"][:])
        # all token indices, partition-major: tok_sb[p, c] = token[c*128+p]
        self.tok_sb = p["wts"].tile([128, 64], I32, name="tok_sb")
        nc.sync.dma_start(self.tok_sb[:], dr["tokens_tr"][:])

        self.pools = p
        self.wx = {}   # pair -> tile [128, 8192] fp16 (f blk at 0, b blk at 4096)

    # ---- production of one pair's wx: (f, pair) and (b, NBLK-1-pair) ----
    def production_items(self, pair):
        """Returns a list of closures emitting the gather/transpose/matmul/
        bias work that materializes wx[pair] (both dirs)."""
        nc, dr, p = self.nc, self.dr, self.pools
        items = []
        xg = {}
        state = {}
        blk_of = {"f": pair, "b": self.NBLK - 1 - pair}

        def alloc():
            state["wx"] = p["wx"].tile([128, 8192], F16, name="wx")
            self.wx[pair] = state["wx"]
            state["xt_f"] = p["xt"].tile([128, 2048], F16, name="xt_f")
            state["xt_b"] = p["xt"].tile([128, 2048], F16, name="xt_b")
            # pad mask for the bwd dir, broadcast to all partitions:
            # -60 at invalid (t, b) in the i/f/o chunks, 0 in the g chunk
            exm = p["extra"].tile([128, 4096], F16, name="exmask")
            nc.sync.dma_start(
                exm[:],
                bass.AP(dr["exmask_b"], blk_of["b"] * 4096,
                        [[0, 128], [1, 4096]]),
            )
            state["exm"] = exm

        def gather(d, i):
            def go():
                c = blk_of[d] * 8 + i
                t = p["xg"].tile([128, 256], F16)
                nc.gpsimd.indirect_dma_start(
                    out=t[:], out_offset=None, in_=dr["emb16"][:],
                    in_offset=bass.IndirectOffsetOnAxis(
                        ap=self.tok_sb[:, c:c + 1], axis=0),
                )
                xg[(d, i)] = t
            return go

        def transp2(d, i):
            def go():
                ps = p["tp_ps"].tile([128, 256], F16)
                for h in range(2):
                    nc.tensor.transpose(
                        out=ps[:, h * 128:(h + 1) * 128],
                        in_=xg[(d, i)][:, h * 128:(h + 1) * 128],
                        identity=self.ident[:],
                    )
                nc.vector.tensor_copy(
                    A(state[f"xt_{d}"], i * 128, [[1, 128], [1024, 2], [1, 128]]),
                    A(ps, 0, [[1, 128], [128, 2], [1, 128]]),
                )
            return go

        def mm(d, j, n, h):
            def go():
                if h == 0:
                    state[("ps", d, j, n)] = p["wx_ps"].tile(
                        [128, 512], F32, name="wx_ps")
                nc.tensor.matmul(
                    out=state[("ps", d, j, n)][:],
                    lhsT=self.wihT[d][h][:, j * 128:(j + 1) * 128],
                    rhs=state[f"xt_{d}"][
                        :, h * 1024 + n * 512: h * 1024 + (n + 1) * 512],
                    start=(h == 0), stop=(h == 1),
                )
            return go

        def bias(d, j, n):
            def go():
                # wx col layout is (tin, j, b): per-slot reads are contiguous
                # 64-col windows and each copy stays inside one half.
                d0 = 0 if d == "f" else 4096
                dst = A(state["wx"], d0 + n * 2048 + j * 16,
                        [[1, 128], [64, 32], [1, 16]])
                if d == "f":
                    # fwd: psum -> sbuf with per-partition bias, on ACT
                    nc.scalar.activation(
                        out=dst, in_=state[("ps", d, j, n)][:],
                        func=AF.Identity, bias=self.biasT[d][:, j:j + 1])
                else:
                    # bwd: psum + bias + pad mask, on DVE
                    nc.vector.scalar_tensor_tensor(
                        out=dst, in0=state[("ps", d, j, n)][:],
                        scalar=self.biasT[d][:, j:j + 1],
                        in1=A(state["exm"], n * 2048 + j * 16,
                              [[1, 128], [64, 32], [1, 16]]),
                        op0=OP.add, op1=OP.add,
                    )
            return go

        # first-needed halves first: f consumes n=0 at slots 0..31 of its
        # block, b consumes n=1 (tin_b 63..32). Gathers i 0..3 feed n=0,
        # i 4..7 feed n=1.
        items.append(alloc)
        halves = [(("f", range(0, 4), 0), ("b", range(4, 8), 1)),
                  (("f", range(4, 8), 1), ("b", range(0, 4), 0))]
        for half in halves:
            for d, irng, n in half:
                for i in irng:
                    items.append(gather(d, i))
            for d, irng, n in half:
                for i in irng:
                    items.append(transp2(d, i))
            for j in range(4):
                for d, irng, n in half:
                    items.append(mm(d, j, n, 0))
                    items.append(mm(d, j, n, 1))
                    items.append(bias(d, j, n))
        return items
    # items in the first half: alloc + 8 gathers + 8 transposes + 24 mm/copy
    PROLOGUE_ITEMS = 41

    # ---- one recurrence slot: fwd step t_f and bwd step t_b merged ----
    def slot(self, t_f):
        nc, p, T = self.nc, self.pools, self.T
        t_b = T - 1 - t_f
        ha = self.hist_all
        hb0 = self.hb_off
        hprev = {"f": ha[:, t_f * 16:(t_f + 1) * 16],
                 "b": ha[:, hb0 + (t_b + 1) * 16:hb0 + (t_b + 2) * 16]}
        tin = t_f % 64
        wxt = self.wx[t_f // 64]
        # gates psum: f at cols 0:64, b at 64:128; one wx load matmul.
        # wx col layout (tin, j, b): slot reads contiguous 64-col windows.
        g_ps = p["g_ps"].tile([128, 128], F32, name="g_ps")
        xstep = 4096 + (63 - tin) * 64 - tin * 64
        nc.tensor.matmul(
            out=g_ps[:],
            lhsT=self.ident[:],
            rhs=A(wxt, tin * 64, [[1, 128], [xstep, 2], [1, 64]]),
            start=True, stop=False,
        )
        for di, d in enumerate(("f", "b")):
            for j in range(4):
                nc.tensor.matmul(
                    out=g_ps[:, di * 64 + j * 16:di * 64 + (j + 1) * 16],
                    lhsT=self.whhT[d][:, j * 128:(j + 1) * 128],
                    rhs=hprev[d], start=False, stop=(di == 1 and j == 3),
                )
        act = p["act"].tile([128, 128], F32, name="act")
        nc.scalar.activation(
            out=A(act, 0, [[1, 128], [64, 2], [1, 48]]),
            in_=A(g_ps, 0, [[1, 128], [64, 2], [1, 48]]), func=AF.Sigmoid)
        c = self.cboth
        m2 = p["m2"].tile([128, 32], F32, name="m2")
        nc.vector.tensor_mul(
            out=m2[:],
            in0=A(act, 16, [[1, 128], [64, 2], [1, 16]]),
            in1=c[:],
        )
        nc.scalar.activation(
            out=A(act, 48, [[1, 128], [64, 2], [1, 16]]),
            in_=A(g_ps, 48, [[1, 128], [64, 2], [1, 16]]), func=AF.Tanh)
        m1 = p["m1"].tile([128, 32], F32, name="m1")
        nc.vector.tensor_mul(
            out=m1[:],
            in0=A(act, 0, [[1, 128], [64, 2], [1, 16]]),
            in1=A(act, 48, [[1, 128], [64, 2], [1, 16]]),
        )
        nc.vector.tensor_add(out=c[:], in0=m1[:], in1=m2[:])
        # tiny PE warm-up matmul, data-dependent on m1 so it fires shortly
        # before the next whh burst and lifts the PE out of its low p-state
        nc.tensor.matmul(out=g_ps[0:16, 0:16], lhsT=self.ident32[:],
                         rhs=m1[:, 0:16], start=True, stop=True)
        tc_ = p["tc2"].tile([128, 32], F32, name="tc2")
        nc.scalar.activation(out=tc_[:], in_=c[:], func=AF.Tanh)
        nc.tensor.matmul(out=g_ps[0:16, 16:32], lhsT=self.ident32[:],
                         rhs=tc_[:, 0:16], start=True, stop=True)
        # merged h write: f dest at (t_f+1)*16, b dest at hb0 + t_b*16
        hstep = hb0 + t_b * 16 - (t_f + 1) * 16
        nc.vector.tensor_mul(
            out=A(ha, (t_f + 1) * 16, [[1, 128], [hstep, 2], [1, 16]]),
            in0=A(act, 32, [[1, 128], [64, 2], [1, 16]]),
            in1=A(tc_, 0, [[1, 128], [16, 2], [1, 16]]),
        )

    # ---- full pipelined emission ---------------------------------------
    def emit_recurrence(self):
        T, NBLK = self.T, self.NBLK
        # prologue: first-needed halves of pair 0 only; rest drips into blk 0
        items0 = self.production_items(0)
        for it in items0[:self.PROLOGUE_ITEMS]:
            it()
        leftover = items0[self.PROLOGUE_ITEMS:]
        for blk in range(NBLK):
            todo = leftover
            leftover = []
            if blk + 1 < NBLK:
                todo = todo + self.production_items(blk + 1)
            k = 0
            for tin in range(64):
                self.slot(blk * 64 + tin)
                want = ((tin + 1) * len(todo)) // 64
                while k < want:
                    todo[k]()
                    k += 1

    def emit_scores(self):
        nc, p, T = self.nc, self.pools, self.T
        NS = T * 16 // 128
        sb = p["sc_sb"].tile([128, 4 * NS], F32)
        for n in range(NS):
            ps = p["sc_ps"].tile([128, 4], F32)
            nc.tensor.matmul(out=ps[:], lhsT=self.hist["f"][:, 16 + n * 128: 16 + (n + 1) * 128],
                             rhs=self.wlabT["f"][:], start=True, stop=False)
            nc.tensor.matmul(out=ps[:], lhsT=self.hist["b"][:, n * 128:(n + 1) * 128],
                             rhs=self.wlabT["b"][:], start=False, stop=True)
            nc.vector.tensor_copy(sb[:, n * 4:(n + 1) * 4], ps[:])
        nc.sync.dma_start(
            bass.AP(self.dr["scores"], 0, [[4, 128], [512, NS], [1, 4]]),
            A(sb, 0, [[1, 128], [4, NS], [1, 4]]),
        )


def build_exmask_b(lens, T):
    import numpy as np
    # exmask_b[blk, c]: -60 at invalid (t, b) for the i/f/o gates, 0 for the
    # g gate; col layout c = tin*64 + j*16 + b.
    NBLK = T // 64
    ev = np.zeros((NBLK, 64, 4, 16), np.float32)
    tin = np.arange(64)
    for blk in range(NBLK):
        t = blk * 64 + tin
        inv = -60.0 * (t[:, None] >= lens[None, :])  # [64, 16]
        ev[blk, :, :3, :] = inv[:, None, :]
    return ev.astype(np.float16).reshape(-1)


def lstm_dram_specs(T=512):
    return [
        ("emb16", [8000, 256], F16), ("tokens_tr", [128, T * 16 // 128], I32),
        ("wihT_f0", [128, 512], F16), ("wihT_f1", [128, 512], F16),
        ("wihT_b0", [128, 512], F16), ("wihT_b1", [128, 512], F16),
        ("whhT_f", [128, 512], F16), ("whhT_b", [128, 512], F16),
        ("biasT_f", [128, 4], F32), ("biasT_b", [128, 4], F32),
        ("exmask_b", [8 * 4096], F16),
        ("wlabT_f", [128, 4], F16), ("wlabT_b", [128, 4], F16),
        ("ident", [128, 128], F16), ("ident32", [128, 16], F32),
    ]


def make_pools(ctx_persist, ctx_trans, tc):
    p = {}
    p["hist"] = ctx_persist.enter_context(tc.tile_pool(name="hist", bufs=1))
    p["state"] = ctx_persist.enter_context(tc.tile_pool(name="state", bufs=1))
    p["wts"] = ctx_persist.enter_context(tc.tile_pool(name="wts", bufs=1))
    p["extra"] = ctx_trans.enter_context(tc.tile_pool(name="extra", bufs=2))
    p["xg"] = ctx_trans.enter_context(tc.tile_pool(name="xg", bufs=16))
    p["xt"] = ctx_trans.enter_context(tc.tile_pool(name="xt", bufs=4))
    p["wx"] = ctx_trans.enter_context(tc.tile_pool(name="wx", bufs=2))
    p["tp_ps"] = ctx_trans.enter_context(tc.tile_pool(name="tp_ps", bufs=2, space="PSUM"))
    p["wx_ps"] = ctx_trans.enter_context(tc.tile_pool(name="wx_ps", bufs=2, space="PSUM"))
    p["g_ps"] = ctx_trans.enter_context(tc.tile_pool(name="g_ps", bufs=2, space="PSUM"))
    for nm in ("act", "m1", "m2", "tc2"):
        p[nm] = ctx_trans.enter_context(tc.tile_pool(name=nm, bufs=2))
    return p


def make_score_pools(ctx, tc):
    p = {}
    p["sc_ps"] = ctx.enter_context(tc.tile_pool(name="sc_ps", bufs=8, space="PSUM"))
    p["sc_sb"] = ctx.enter_context(tc.tile_pool(name="sc_sb", bufs=1))
    return p


# ---------------------------------------------------------------------------
# DRAM declarations + host prep + SPMD driver
# ---------------------------------------------------------------------------

def _build_program():
    nc = bass.Bass(trn_type="TRN2")
    dr = {}
    for name, shape, dt in lstm_dram_specs(T):
        dr[name] = nc.dram_tensor(name, shape, dt, kind="ExternalInput")
    for name, shape, dt in CRF_DRAM_SPECS:
        dr[name] = nc.dram_tensor(name, shape, dt, kind="ExternalInput")
    for name, shape, dt in CRF_SCRATCH_SPECS:
        dr[name] = nc.dram_tensor(name, shape, dt)
    dr["scores"] = nc.dram_tensor("scores", [T * 16, 4], F32)
    dr["labels"] = nc.dram_tensor("labels", [NB, T], I32, kind="ExternalOutput")

    with tile.TileContext(nc) as tc:
        with ExitStack() as ctx:
            with ExitStack() as ctx_trans:
                pools = make_pools(ctx, ctx_trans, tc)
                em = LstmEmitter(nc, tc, dr, T, pools)
                em.emit_recurrence()
            spools = make_score_pools(ctx, tc)
            pools.update(spools)
            em.emit_scores()
            with ExitStack() as ctx_crf:
                crf_pool = ctx_crf.enter_context(tc.tile_pool(name="crf", bufs=1))
                emit_crf(nc, tc, dr, crf_pool)
    return nc


_CACHE = {}
LAST_EXEC_NS = None


def kernel(**inputs):
    global LAST_EXEC_NS
    _apply_patches()
    from concourse.bass_utils import run_bass_kernel_spmd

    inp = {k: np.asarray(v) for k, v in inputs.items()}
    if "nc" not in _CACHE:
        _CACHE["nc"] = _build_program()
    nc = _CACHE["nc"]

    # shared (batch-independent) host arrays
    shared = {}
    shared["emb16"] = inp["emb"].astype(np.float16)
    perm = np.concatenate([np.arange(128), 128 + np.arange(128),
                           384 + np.arange(128), 256 + np.arange(128)])
    for d, sfx in (("f", "_f"), ("b", "_b")):
        wih = inp[f"W_ih{sfx}"][perm].astype(np.float32)
        whh = inp[f"W_hh{sfx}"][perm].astype(np.float32)
        bias = (inp[f"b_ih{sfx}"] + inp[f"b_hh{sfx}"])[perm].astype(np.float32)
        shared[f"wihT_{d}0"] = np.ascontiguousarray(wih.T[:128]).astype(np.float16)
        shared[f"wihT_{d}1"] = np.ascontiguousarray(wih.T[128:]).astype(np.float16)
        shared[f"whhT_{d}"] = np.ascontiguousarray(whh.T).astype(np.float16)
        shared[f"biasT_{d}"] = np.ascontiguousarray(
            bias.reshape(4, 128).T).astype(np.float32)

        wl = inp["W_lab"].astype(np.float32)
        half = wl[:, :128] if d == "f" else wl[:, 128:]
        shared[f"wlabT_{d}"] = np.ascontiguousarray(half.T).astype(np.float16)
    shared["ident"] = np.eye(128, dtype=np.float16)
    shared["ident32"] = np.eye(128, 16, dtype=np.float32)

    trans = inp["transitions"].astype(np.float32)
    fromB = inp["from_BOS"].astype(np.float32)
    toEOS = inp["to_EOS"].astype(np.float32)
    b_lab = inp["b_lab"].astype(np.float32)

    pad_seq = inp["pad_seq"].astype(np.int64)
    lens_full = inp["lens"].astype(np.int64)

    in_maps = []
    for core in range(NCORES):
        b0 = core * NB
        seq = pad_seq[b0:b0 + NB]
        lens = lens_full[b0:b0 + NB]
        m = dict(shared)
        tok = np.ascontiguousarray(seq.T).reshape(-1).astype(np.int32)
        m["tokens_tr"] = np.ascontiguousarray(tok.reshape(T * 16 // 128, 128).T)
        m["exmask_b"] = build_exmask_b(lens, T)
        m.update(host_crf_consts(lens, trans, fromB, toEOS, b_lab))
        in_maps.append(m)

    res = run_bass_kernel_spmd(nc, in_maps, list(range(NCORES)))
    LAST_EXEC_NS = res.exec_time_ns
    out = np.concatenate([res.results[c]["labels"] for c in range(NCORES)], axis=0)
    return out.astype(np.int32)



# revision 60
# speedup vs baseline: 1.0447x; 1.0447x over previous
"""BiLSTM-CRF Trainium2 kernel (Bass/Tile), data-parallel over batch on 8
NeuronCores. Self-contained: host prep + device emission + SPMD runner.

Pipeline per core (16 sequences, T=512):
  embedding gather (indirect DMA, fp16) -> PE transpose -> Wx matmuls (fp16),
  software-pipelined with the serial BiLSTM recurrence (gates on partitions,
  fp16 weights/hist, fp32 cell state) -> emission scores matmul -> blocked
  Viterbi forward scan + blocked backtrace (max-plus / one-hot map composition
  in 32 chunks of 16 steps, vectorized across 128 partitions).
"""
import sys
import types
import numpy as np

import concourse.bass as bass
import concourse.mybir as mybir
from concourse import tile
from concourse.vector_clock import ScopedClock
import bass_rust
from contextlib import ExitStack

F16 = mybir.dt.float16
F32 = mybir.dt.float32
I32 = mybir.dt.int32
AF = mybir.ActivationFunctionType
AX = mybir.AxisListType.X
OP = mybir.AluOpType

B_FULL, T, V, D = 128, 512, 8000, 256
NB = 16          # sequences per core
NCORES = 8


# ---------------------------------------------------------------------------
# Harness workarounds: walrus in this environment accepts only ONE sync-wait
# per instruction; split extras onto NoOps (BIR json pass) and chunk the Tile
# exit drain. Also register the NTFF profile hook shim so BASS_TRACE=1 works.
# ---------------------------------------------------------------------------
import json as _json

_SW_CTR = [0]


def _split_sync_waits(bir_json: bytes) -> bytes:
    d = _json.loads(bir_json)
    changed = False
    for fn in d.get("functions", []):
        for blk in fn.get("blocks", []):
            new_insts = []
            for inst in blk.get("instructions", []):
                si = inst.get("sync_info")
                waits = (si or {}).get("on_wait") or []
                if len(waits) > 1:
                    changed = True
                    for w in waits[:-1]:
                        _SW_CTR[0] += 1
                        nop = {
                            "engine": inst["engine"],
                            "ins": [],
                            "outs": [],
                            "name": f"I-swsplit-{_SW_CTR[0]}",
                            "opcode": "NoOp",
                            "sync_info": {"on_update": [], "on_wait": [w]},
                        }
                        if "debug" in inst:
                            nop["debug"] = inst["debug"]
                        new_insts.append(nop)
                    si["on_wait"] = [waits[-1]]
                new_insts.append(inst)
            blk["instructions"] = new_insts
    return _json.dumps(d).encode() if changed else bir_json


def _patched_drain_and_barrier(self, tick_clock, wait_clock):
    drain_inst = self.nc.sync.drain()
    wait_clock.add_sem_waits(
        drain_inst.ins, ScopedClock({None: tick_clock.global_clock})
    )
    si = drain_inst.ins.sync_info
    if si is not None and si.on_wait is not None and len(si.on_wait) > 1:
        waits = list(si.on_wait)
        drain_inst.ins.sync_info = bass_rust.SyncInfo(
            on_wait=waits[:1], on_update=list(si.on_update or [])
        )
        for i in range(1, len(waits)):
            nop = self.nc.sync.nop()
            nop.ins.sync_info = bass_rust.SyncInfo(on_wait=[waits[i]], on_update=[])
    self.nc.all_engine_barrier()
    assert self.sems is not None
    popped = self.nc._tile_sem_poison_stack.pop()
    assert popped is self._sem_poison
    self.nc.clear_and_free_semaphores(list(self.sems.allocated().values()))
    self.nc.all_engine_barrier()


_PATCHED = [False]


def _apply_patches():
    if _PATCHED[0]:
        return
    _PATCHED[0] = True
    tile.TileContext._drain_and_barrier = _patched_drain_and_barrier
    import concourse.bass_utils as _bu
    import concourse.bass2jax as _b2j

    _orig_compile = _bu.compile_bir_kernel

    def _wrapped(bir_json, tmpdir, neff_name="file.neff"):
        return _orig_compile(_split_sync_waits(bir_json), tmpdir, neff_name)

    _wrapped._swsplit_wrapped = True
    _bu.compile_bir_kernel = _wrapped
    _b2j.compile_bir_kernel = _wrapped

    if "antenv.axon_hooks" not in sys.modules:
        try:
            import trn_agent_boot.trn_boot as _tb
            _hook = _tb._ntff_profile_via_ctypes("/opt/axon/libaxon_pjrt.so")
        except Exception:
            _hook = None
        m = types.ModuleType("antenv.axon_hooks")
        m.get_axon_ntff_profile_hook = lambda: _hook
        m.set_axon_ntff_profile_hook = lambda h: None
        sys.modules["antenv.axon_hooks"] = m





def A(t, off, dims, p0=0):
    # t: pool tile AP [[rowsize, P], [1, rowsize]]. dims[0] is the partition
    # pair whose step is replaced by the tile's canonical per-partition row
    # size; off is the within-partition element offset.
    rs = t.ap[0][0]
    d = [list(x) for x in dims]
    d[0] = [rs, d[0][1]]
    return bass.AP(t.tensor, t.offset + p0 * rs + off, d)


def AD(handle, off, dims):
    return bass.AP(handle, off, [list(d) for d in dims])


def emit_crf(nc, tc, dr, pool):
    """dr: dict of DRAM handles. pool: sbuf tile pool to allocate from."""
    v = nc.vector

    # ---- V0: build T matrices ------------------------------------------
    scT = pool.tile([128, 256], F32)   # (chpos, s, c)
    # scores_dram is tok-major [8192, 4]: addr = (t*16+b)*4 + c
    for g in range(8):
        nc.sync.dma_start(
            A(scT, 0, [[1, 16], [4, 64], [1, 4]], p0=g * 16),
            AD(dr["scores"], g * 4096, [[4, 16], [64, 64], [1, 4]]),
        )
    transb_sb = pool.tile([128, 16], F32)
    nc.sync.dma_start(transb_sb[:], dr["transb16"][None, :].to_broadcast((128, 16)))
    imp_sb = pool.tile([128, 16], F32)
    nc.sync.dma_start(imp_sb[:], dr["impflat"][None, :].to_broadcast((128, 16)))
    vmask_sb = pool.tile([128, 1024], I32)
    nc.sync.dma_start(vmask_sb[:], dr["vmask128"][:])
    fromBp_sb = pool.tile([128, 4], F32)
    nc.sync.dma_start(fromBp_sb[:], dr["fromBp4"][None, :].to_broadcast((128, 4)))

    Traw = pool.tile([128, 1024], F32)  # (chpos, s, p, c)
    v.tensor_add(
        out=A(Traw, 0, [[1, 128], [256, 4], [16, 16], [4, 4], [1, 4]]),
        in0=A(scT, 0, [[1, 128], [64, 4], [4, 16], [0, 4], [1, 4]]),
        in1=A(transb_sb, 0, [[1, 128], [0, 4], [0, 16], [4, 4], [1, 4]]),
    )
    T128 = pool.tile([128, 1024], F32)
    v.select(
        out=A(T128, 0, [[1, 128], [256, 4], [16, 16], [4, 4], [1, 4]]),
        mask=A(vmask_sb, 0, [[1, 128], [256, 4], [16, 16], [4, 4], [1, 4]]),
        on_true=A(Traw, 0, [[1, 128], [256, 4], [16, 16], [4, 4], [1, 4]]),
        on_false=A(imp_sb, 0, [[1, 128], [0, 4], [0, 16], [4, 4], [1, 4]]),
    )
    # step 0 (partitions 0:16, chpos=0, s=0): T = e0 + fromBp (rows equal)
    v.tensor_add(
        out=A(T128, 0, [[1, 16], [4, 4], [1, 4]]),
        in0=A(scT, 0, [[1, 16], [0, 4], [1, 4]]),
        in1=A(fromBp_sb, 0, [[1, 16], [0, 4], [1, 4]]),
    )

    # ---- V1: chunk max-plus products -----------------------------------
    Ma = pool.tile([128, 64], F32)   # (chpos, i, k/j)
    Mb = pool.tile([128, 64], F32)
    tmp256 = pool.tile([128, 1024], F32)
    v.tensor_copy(
        A(Ma, 0, [[1, 128], [16, 4], [4, 4], [1, 4]]),
        A(T128, 0, [[1, 128], [256, 4], [4, 4], [1, 4]]),
    )
    cur, nxt = Ma, Mb
    for s in range(1, 16):
        v.tensor_add(
            out=A(tmp256, 0, [[1, 128], [64, 4], [16, 4], [4, 4], [1, 4]]),
            in0=A(cur, 0, [[1, 128], [16, 4], [4, 4], [1, 4], [0, 4]]),
            in1=A(T128, s * 16, [[1, 128], [256, 4], [0, 4], [4, 4], [1, 4]]),
        )
        v.tensor_reduce(
            out=A(nxt, 0, [[1, 128], [16, 4], [4, 4], [1, 4]]),
            in_=A(tmp256, 0, [[1, 128], [64, 4], [16, 4], [1, 4], [4, 4]]),
            axis=AX, op=OP.max,
        )
        cur, nxt = nxt, cur
    nc.sync.dma_start(
        AD(dr["mdram"], 0, [[64, 128], [1, 64]]),
        A(cur, 0, [[1, 128], [1, 64]]),
    )

    # ---- V2: serial chunk scan (16 partitions) -------------------------
    M16 = pool.tile([16, 512], F32)
    for g in range(8):
        nc.sync.dma_start(
            A(M16, g * 64, [[1, 16], [1, 64]]),
            AD(dr["mdram"], g * 1024, [[64, 16], [1, 64]]),
        )
    Ball = pool.tile([16, 132], F32)
    v.memset(Ball[:], 0.0)
    t16 = pool.tile([16, 16], F32)
    for c in range(32):
        v.tensor_add(
            out=A(t16, 0, [[1, 16], [4, 4], [1, 4]]),
            in0=A(Ball, c * 4, [[1, 16], [1, 4], [0, 4]]),
            in1=A(M16, c * 16, [[1, 16], [4, 4], [1, 4]]),
        )
        v.tensor_reduce(
            out=A(Ball, (c + 1) * 4, [[1, 16], [1, 4]]),
            in_=A(t16, 0, [[1, 16], [1, 4], [4, 4]]),
            axis=AX, op=OP.max,
        )
    # last label one-hot
    toEOS_sb = pool.tile([16, 4], F32)
    nc.sync.dma_start(toEOS_sb[:], dr["toEOS4"][None, :].to_broadcast((16, 4)))
    c3lab_sb = pool.tile([16, 4], F32)
    nc.sync.dma_start(c3lab_sb[:], dr["c3lab4"][None, :].to_broadcast((16, 4)))
    wiota16 = pool.tile([16, 4], F32)
    nc.sync.dma_start(wiota16[:], dr["wiota4"][None, :].to_broadcast((16, 4)))
    fin = pool.tile([16, 4], F32)
    v.tensor_add(out=fin[:], in0=A(Ball, 128, [[1, 16], [1, 4]]), in1=toEOS_sb[:])
    lmax = pool.tile([16, 1], F32)
    v.tensor_reduce(out=lmax[:], in_=fin[:], axis=AX, op=OP.max)
    loh = pool.tile([16, 4], F32)
    v.tensor_tensor(out=loh[:], in0=fin[:],
                    in1=A(lmax, 0, [[1, 16], [0, 4]]), op=OP.is_equal)
    lohm = pool.tile([16, 4], F32)
    v.tensor_mul(out=lohm[:], in0=loh[:], in1=c3lab_sb[:])
    lenc = pool.tile([16, 1], F32)
    v.tensor_reduce(out=lenc[:], in_=lohm[:], axis=AX, op=OP.max)
    llval = pool.tile([16, 1], F32)
    v.tensor_scalar(out=llval[:], in0=lenc[:], scalar1=-1.0, scalar2=3.0,
                    op0=OP.mult, op1=OP.add)
    lloh = pool.tile([16, 4], F32)
    v.tensor_tensor(out=lloh[:], in0=wiota16[:],
                    in1=A(llval, 0, [[1, 16], [0, 4]]), op=OP.is_equal)
    nc.sync.dma_start(AD(dr["lldram"], 0, [[4, 16], [1, 4]]), lloh[:])
    nc.sync.dma_start(AD(dr["edram"], 0, [[132, 16], [1, 132]]), Ball[:])

    # ---- V3: replay -> backtrace tables --------------------------------
    c3p_sb = pool.tile([128, 16], F32)
    nc.sync.dma_start(c3p_sb[:], dr["c3p16"][None, :].to_broadcast((128, 16)))
    bestA = pool.tile([128, 16], F32)
    bestB = pool.tile([128, 16], F32)
    nc.sync.dma_start(
        bestA[:], AD(dr["edram"], 0, [[16, 8], [132, 16], [4, 4], [1, 4]])
    )
    BT = pool.tile([128, 256], F32)     # (chpos, s, c)
    smat = pool.tile([128, 64], F32)
    oh64 = pool.tile([128, 64], F32)
    enc128 = pool.tile([128, 16], F32)
    bcur, bnxt = bestA, bestB
    for s in range(16):
        v.tensor_add(
            out=A(smat, 0, [[1, 128], [16, 4], [4, 4], [1, 4]]),
            in0=A(bcur, 0, [[1, 128], [4, 4], [1, 4], [0, 4]]),
            in1=A(T128, s * 16, [[1, 128], [256, 4], [4, 4], [1, 4]]),
        )
        v.tensor_reduce(
            out=A(bnxt, 0, [[1, 128], [4, 4], [1, 4]]),
            in_=A(smat, 0, [[1, 128], [16, 4], [1, 4], [4, 4]]),
            axis=AX, op=OP.max,
        )
        v.tensor_tensor(
            out=A(oh64, 0, [[1, 128], [16, 4], [4, 4], [1, 4]]),
            in0=A(smat, 0, [[1, 128], [16, 4], [4, 4], [1, 4]]),
            in1=A(bnxt, 0, [[1, 128], [4, 4], [0, 4], [1, 4]]),
            op=OP.is_equal,
        )
        v.tensor_mul(
            out=A(oh64, 0, [[1, 128], [16, 4], [4, 4], [1, 4]]),
            in0=A(oh64, 0, [[1, 128], [16, 4], [4, 4], [1, 4]]),
            in1=A(c3p_sb, 0, [[1, 128], [0, 4], [4, 4], [1, 4]]),
        )
        v.tensor_reduce(
            out=A(enc128, 0, [[1, 128], [4, 4], [1, 4]]),
            in_=A(oh64, 0, [[1, 128], [16, 4], [1, 4], [4, 4]]),
            axis=AX, op=OP.max,
        )
        v.tensor_scalar(
            out=A(BT, s * 4, [[1, 128], [64, 4], [1, 4]]),
            in0=A(enc128, 0, [[1, 128], [4, 4], [1, 4]]),
            scalar1=-1.0, scalar2=3.0, op0=OP.mult, op1=OP.add,
        )
        bcur, bnxt = bnxt, bcur
    nc.sync.dma_start(
        AD(dr["btdram"], 0, [[256, 128], [1, 256]]),
        A(BT, 0, [[1, 128], [1, 256]]),
    )

    # ---- VA: backtrace map tables + chunk compositions -----------------
    BTS = pool.tile([128, 256], F32)
    # top group's last slot is never used; zero-fill before partial overwrite
    v.memset(A(BTS, 252, [[1, 128], [1, 4]]), 0.0)
    nc.sync.dma_start(
        A(BTS, 0, [[1, 128], [1, 252]]),
        AD(dr["btdram"], 4, [[256, 128], [1, 252]]),
    )
    # last slot of each partition: first bt entry of the next chunk group
    nc.sync.dma_start(
        A(BTS, 252, [[1, 112], [1, 4]]),
        AD(dr["btdram"], 16 * 256, [[256, 112], [1, 4]]),
    )
    meq_sb = pool.tile([128, 64], I32)
    mlt_sb = pool.tile([128, 64], I32)
    nc.sync.dma_start(meq_sb[:], dr["meq128"][:])
    nc.sync.dma_start(mlt_sb[:], dr["mlt128"][:])
    lloh128 = pool.tile([128, 4], F32)
    nc.sync.dma_start(lloh128[:], AD(dr["lldram"], 0, [[0, 8], [4, 16], [1, 4]]))
    i4_sb = pool.tile([128, 16], F32)
    nc.sync.dma_start(i4_sb[:], dr["i4flat"][None, :].to_broadcast((128, 16)))
    wiota128 = pool.tile([128, 4], F32)
    nc.sync.dma_start(wiota128[:], dr["wiota4"][None, :].to_broadcast((128, 4)))

    Fall = pool.tile([128, 1024], F32)  # (chpos, s, u, w)
    tmpA = pool.tile([128, 64], F32)
    for s in range(16):
        # oh(u,w) = bt_{t+1}[u] == w
        v.tensor_tensor(
            out=A(tmpA, 0, [[1, 128], [16, 4], [4, 4], [1, 4]]),
            in0=A(BTS, s * 4, [[1, 128], [64, 4], [1, 4], [0, 4]]),
            in1=A(wiota128, 0, [[1, 128], [0, 4], [0, 4], [1, 4]]),
            op=OP.is_equal,
        )
        # tmp2 = meq ? lloh : I4  ; F = mlt ? oh : tmp2  (write into Fall)
        v.select(
            out=A(Fall, s * 16, [[1, 128], [256, 4], [4, 4], [1, 4]]),
            mask=A(meq_sb, s, [[1, 128], [16, 4], [0, 4], [0, 4]]),
            on_true=A(lloh128, 0, [[1, 128], [0, 4], [0, 4], [1, 4]]),
            on_false=A(i4_sb, 0, [[1, 128], [0, 4], [4, 4], [1, 4]]),
        )
        v.select(
            out=A(Fall, s * 16, [[1, 128], [256, 4], [4, 4], [1, 4]]),
            mask=A(mlt_sb, s, [[1, 128], [16, 4], [0, 4], [0, 4]]),
            on_true=A(tmpA, 0, [[1, 128], [16, 4], [4, 4], [1, 4]]),
            on_false=A(Fall, s * 16, [[1, 128], [256, 4], [4, 4], [1, 4]]),
        )
    Ga = pool.tile([128, 64], F32)
    Gb = pool.tile([128, 64], F32)
    v.tensor_copy(
        A(Ga, 0, [[1, 128], [16, 4], [4, 4], [1, 4]]),
        A(Fall, 15 * 16, [[1, 128], [256, 4], [4, 4], [1, 4]]),
    )
    gcur, gnxt = Ga, Gb
    for s in range(14, -1, -1):
        v.tensor_mul(
            out=A(tmp256, 0, [[1, 128], [64, 4], [16, 4], [4, 4], [1, 4]]),
            in0=A(gcur, 0, [[1, 128], [16, 4], [4, 4], [1, 4], [0, 4]]),
            in1=A(Fall, s * 16, [[1, 128], [256, 4], [0, 4], [4, 4], [1, 4]]),
        )
        v.tensor_reduce(
            out=A(gnxt, 0, [[1, 128], [16, 4], [4, 4], [1, 4]]),
            in_=A(tmp256, 0, [[1, 128], [64, 4], [16, 4], [1, 4], [4, 4]]),
            axis=AX, op=OP.max,
        )
        gcur, gnxt = gnxt, gcur
    nc.sync.dma_start(
        AD(dr["gdram"], 0, [[64, 128], [1, 64]]),
        A(gcur, 0, [[1, 128], [1, 64]]),
    )

    # ---- VB: serial reverse chunk scan (16 partitions) -----------------
    Gall16 = pool.tile([16, 512], F32)
    for g in range(8):
        nc.sync.dma_start(
            A(Gall16, g * 64, [[1, 16], [1, 64]]),
            AD(dr["gdram"], g * 1024, [[64, 16], [1, 64]]),
        )
    EB = pool.tile([16, 132], F32)
    nc.sync.dma_start(
        A(EB, 128, [[1, 16], [1, 4]]), dr["e0oh4"][None, :].to_broadcast((16, 4))
    )
    tb16 = pool.tile([16, 16], F32)
    for c in range(31, -1, -1):
        v.tensor_mul(
            out=tb16[:],
            in0=A(EB, (c + 1) * 4, [[1, 16], [1, 4], [0, 4]]),
            in1=A(Gall16, c * 16, [[1, 16], [4, 4], [1, 4]]),
        )
        v.tensor_reduce(
            out=A(EB, c * 4, [[1, 16], [1, 4]]),
            in_=A(tb16, 0, [[1, 16], [1, 4], [4, 4]]),
            axis=AX, op=OP.max,
        )
    nc.sync.dma_start(AD(dr["ebdram"], 0, [[132, 16], [1, 132]]), EB[:])

    # ---- VC: labels -----------------------------------------------------
    cohE = pool.tile([128, 16], F32)
    nc.sync.dma_start(
        cohE[:], AD(dr["ebdram"], 4, [[16, 8], [132, 16], [4, 4], [1, 4]])
    )
    LABOH = pool.tile([128, 256], F32)  # (chpos, s, w)
    tmpc = pool.tile([128, 64], F32)
    for s in range(15, -1, -1):
        if s == 15:
            in0 = A(cohE, 0, [[1, 128], [4, 4], [1, 4], [0, 4]])
        else:
            in0 = A(LABOH, (s + 1) * 4, [[1, 128], [64, 4], [1, 4], [0, 4]])
        v.tensor_mul(
            out=A(tmpc, 0, [[1, 128], [16, 4], [4, 4], [1, 4]]),
            in0=in0,
            in1=A(Fall, s * 16, [[1, 128], [256, 4], [4, 4], [1, 4]]),
        )
        v.tensor_reduce(
            out=A(LABOH, s * 4, [[1, 128], [64, 4], [1, 4]]),
            in_=A(tmpc, 0, [[1, 128], [16, 4], [1, 4], [4, 4]]),
            axis=AX, op=OP.max,
        )
    omask_sb = pool.tile([128, 64], F32)
    nc.sync.dma_start(omask_sb[:], dr["outmask128"][:])
    labv = pool.tile([128, 64], F32)
    tmpl = pool.tile([128, 256], F32)
    v.tensor_mul(
        out=A(tmpl, 0, [[1, 128], [64, 4], [4, 16], [1, 4]]),
        in0=A(LABOH, 0, [[1, 128], [64, 4], [4, 16], [1, 4]]),
        in1=A(wiota128, 0, [[1, 128], [0, 4], [0, 16], [1, 4]]),
    )
    v.tensor_reduce(
        out=A(labv, 0, [[1, 128], [16, 4], [1, 16]]),
        in_=A(tmpl, 0, [[1, 128], [64, 4], [4, 16], [1, 4]]),
        axis=AX, op=OP.add,
    )
    v.tensor_mul(out=labv[:], in0=labv[:], in1=omask_sb[:])
    labi = pool.tile([128, 64], I32)
    v.tensor_copy(labi[:], labv[:])
    for cp in range(4):
        nc.sync.dma_start(
            AD(dr["labels"], 16 * cp, [[64, 8], [512, 16], [1, 16]]),
            A(labi, cp * 16, [[1, 128], [1, 16]]),
        )


def host_crf_consts(lens, trans, fromB, toEOS, b_lab):
    """All host-side constant arrays, keyed to match dram handle names."""
    import numpy as np
    T, B, L = 512, 16, 4
    NEG = -1e9
    chgrp = np.arange(8)
    out = {}
    out["transb16"] = (trans + b_lab[None, :]).astype(np.float32).reshape(16)
    imp = np.full((L, L), NEG, np.float32)
    np.fill_diagonal(imp, 0.0)
    out["impflat"] = imp.reshape(16)
    out["fromBp4"] = (fromB + b_lab).astype(np.float32)
    out["toEOS4"] = toEOS.astype(np.float32)
    out["c3lab4"] = (3.0 - np.arange(4)).astype(np.float32)
    out["wiota4"] = np.arange(4).astype(np.float32)
    out["c3p16"] = np.repeat(3.0 - np.arange(4), 4).astype(np.float32)
    out["i4flat"] = np.eye(4, dtype=np.float32).reshape(16)
    out["e0oh4"] = np.array([1, 0, 0, 0], np.float32)
    # t value at (P, chpos, s):  P = chgrp*16 + b ; t = 16*(4*chgrp+chpos)+s
    P_chgrp = np.arange(128) // 16
    P_b = np.arange(128) % 16
    chpos = np.arange(4)
    s = np.arange(16)
    tt = 16 * (4 * P_chgrp[:, None, None] + chpos[None, :, None]) + s[None, None, :]
    lb = lens[P_b][:, None, None]
    vm = (tt < lb)
    out["vmask128"] = np.repeat(
        vm.reshape(128, 64)[:, :, None], 16, axis=2
    ).reshape(128, 1024).astype(np.int32)
    out["meq128"] = (tt == lb - 1).reshape(128, 64).astype(np.int32)
    out["mlt128"] = (tt < lb - 1).reshape(128, 64).astype(np.int32)
    out["outmask128"] = (tt < lb).reshape(128, 64).astype(np.float32)
    return out


CRF_DRAM_SPECS = [
    ("transb16", [16], F32), ("impflat", [16], F32), ("fromBp4", [4], F32),
    ("toEOS4", [4], F32), ("c3lab4", [4], F32), ("wiota4", [4], F32),
    ("c3p16", [16], F32), ("i4flat", [16], F32), ("e0oh4", [4], F32),
    ("vmask128", [128, 1024], I32), ("meq128", [128, 64], I32),
    ("mlt128", [128, 64], I32), ("outmask128", [128, 64], F32),
]
CRF_SCRATCH_SPECS = [
    ("mdram", [8192], F32), ("edram", [2112], F32), ("btdram", [32832], F32),
    ("gdram", [8192], F32), ("lldram", [64], F32), ("ebdram", [2112], F32),
]






class LstmEmitter:
    def __init__(self, nc, tc, dr, T, pools):
        self.nc, self.tc, self.dr, self.T = nc, tc, dr, T
        self.NBLK = T // 64
        p = pools
        hist_all = p["hist"].tile([128, 2 * (T + 1) * 16], F16, name="hist_all")
        self.hist_all = hist_all
        self.hb_off = (T + 1) * 16
        self.hist = {
            "f": hist_all[:, 0:(T + 1) * 16],
            "b": hist_all[:, (T + 1) * 16:2 * (T + 1) * 16],
        }
        nc.vector.memset(self.hist["f"][:, 0:16], 0.0)
        nc.vector.memset(self.hist["b"][:, T * 16:(T + 1) * 16], 0.0)
        self.cboth = p["state"].tile([128, 32], F32, name="cboth")
        nc.vector.memset(self.cboth[:], 0.0)
        # weights
        self.whhT = {}
        self.wihT = {}
        self.biasT = {}
        self.wlabT = {}
        for d in ("f", "b"):
            w = p["wts"].tile([128, 512], F16, name=f"whh_{d}")
            nc.sync.dma_start(w[:], dr[f"whhT_{d}"][:])
            self.whhT[d] = w
            hs = []
            for h in range(2):
                wh = p["wts"].tile([128, 512], F16, name=f"wih_{d}{h}")
                nc.sync.dma_start(wh[:], dr[f"wihT_{d}{h}"][:])
                hs.append(wh)
            self.wihT[d] = hs
            bt = p["wts"].tile([128, 4], F32, name=f"bias_{d}")
            nc.sync.dma_start(bt[:], dr[f"biasT_{d}"][:])
            self.biasT[d] = bt
            wl = p["wts"].tile([128, 4], F16, name=f"wlab_{d}")
            nc.sync.dma_start(wl[:], dr[f"wlabT_{d}"][:])
            self.wlabT[d] = wl
        self.ident = p["wts"].tile([128, 128], F16)
        nc.sync.dma_start(self.ident[:], dr["ident"][:])
        self.ident32 = p["wts"].tile([128, 16], F32, name="ident32")
        nc.sync.dma_start(self.ident32[:], dr["ident32"][:])
        self.scr_ps = p["warm_ps"].tile([128, 16], F32, name="scr_ps")
        # all token indices, partition-major: tok_sb[p, c] = token[c*128+p]
        self.tok_sb = p["wts"].tile([128, 64], I32, name="tok_sb")
        nc.sync.dma_start(self.tok_sb[:], dr["tokens_tr"][:])

        self.pools = p
        self.wx = {}   # pair -> tile [128, 8192] fp16 (f blk at 0, b blk at 4096)

    # ---- production of one pair's wx: (f, pair) and (b, NBLK-1-pair) ----
    def production_items(self, pair):
        """Returns a list of closures emitting the gather/transpose/matmul/
        bias work that materializes wx[pair] (both dirs)."""
        nc, dr, p = self.nc, self.dr, self.pools
        items = []
        xg = {}
        state = {}
        blk_of = {"f": pair, "b": self.NBLK - 1 - pair}

        def alloc():
            state["wx"] = p["wx"].tile([128, 8192], F16, name="wx")
            self.wx[pair] = state["wx"]
            state["xt_f"] = p["xt"].tile([128, 2048], F16, name="xt_f")
            state["xt_b"] = p["xt"].tile([128, 2048], F16, name="xt_b")
            # pad mask for the bwd dir, broadcast to all partitions:
            # -60 at invalid (t, b) in the i/f/o chunks, 0 in the g chunk
            exm = p["extra"].tile([128, 4096], F16, name="exmask")
            nc.sync.dma_start(
                exm[:],
                bass.AP(dr["exmask_b"], blk_of["b"] * 4096,
                        [[0, 128], [1, 4096]]),
            )
            state["exm"] = exm

        def gather(d, i):
            def go():
                c = blk_of[d] * 8 + i
                t = p["xg"].tile([128, 256], F16)
                nc.gpsimd.indirect_dma_start(
                    out=t[:], out_offset=None, in_=dr["emb16"][:],
                    in_offset=bass.IndirectOffsetOnAxis(
                        ap=self.tok_sb[:, c:c + 1], axis=0),
                )
                xg[(d, i)] = t
            return go

        def transp2(d, i):
            def go():
                ps = p["tp_ps"].tile([128, 256], F16)
                for h in range(2):
                    nc.tensor.transpose(
                        out=ps[:, h * 128:(h + 1) * 128],
                        in_=xg[(d, i)][:, h * 128:(h + 1) * 128],
                        identity=self.ident[:],
                    )
                nc.vector.tensor_copy(
                    A(state[f"xt_{d}"], i * 128, [[1, 128], [1024, 2], [1, 128]]),
                    A(ps, 0, [[1, 128], [128, 2], [1, 128]]),
                )
            return go

        def mm(d, j, n, h):
            def go():
                if h == 0:
                    state[("ps", d, j, n)] = p["wx_ps"].tile(
                        [128, 512], F32, name="wx_ps")
                nc.tensor.matmul(
                    out=state[("ps", d, j, n)][:],
                    lhsT=self.wihT[d][h][:, j * 128:(j + 1) * 128],
                    rhs=state[f"xt_{d}"][
                        :, h * 1024 + n * 512: h * 1024 + (n + 1) * 512],
                    start=(h == 0), stop=(h == 1),
                )
            return go

        def bias(d, j, n):
            def go():
                # wx col layout is (tin, j, b): per-slot reads are contiguous
                # 64-col windows and each copy stays inside one half.
                d0 = 0 if d == "f" else 4096
                dst = A(state["wx"], d0 + n * 2048 + j * 16,
                        [[1, 128], [64, 32], [1, 16]])
                if d == "f":
                    # fwd: psum -> sbuf with per-partition bias, on ACT
                    nc.scalar.activation(
                        out=dst, in_=state[("ps", d, j, n)][:],
                        func=AF.Identity, bias=self.biasT[d][:, j:j + 1])
                else:
                    # bwd: psum + bias + pad mask, on DVE
                    nc.vector.scalar_tensor_tensor(
                        out=dst, in0=state[("ps", d, j, n)][:],
                        scalar=self.biasT[d][:, j:j + 1],
                        in1=A(state["exm"], n * 2048 + j * 16,
                              [[1, 128], [64, 32], [1, 16]]),
                        op0=OP.add, op1=OP.add,
                    )
            return go

        # first-needed halves first: f consumes n=0 at slots 0..31 of its
        # block, b consumes n=1 (tin_b 63..32). Gathers i 0..3 feed n=0,
        # i 4..7 feed n=1.
        items.append(alloc)
        halves = [(("f", range(0, 4), 0), ("b", range(4, 8), 1)),
                  (("f", range(4, 8), 1), ("b", range(0, 4), 0))]
        for half in halves:
            for d, irng, n in half:
                for i in irng:
                    items.append(gather(d, i))
            for d, irng, n in half:
                for i in irng:
                    items.append(transp2(d, i))
            for j in range(4):
                for d, irng, n in half:
                    items.append(mm(d, j, n, 0))
                    items.append(mm(d, j, n, 1))
                    items.append(bias(d, j, n))
        return items
    # items in the first half: alloc + 8 gathers + 8 transposes + 24 mm/copy
    PROLOGUE_ITEMS = 41

    # ---- one recurrence slot: fwd step t_f and bwd step t_b merged ----
    def slot(self, t_f):
        nc, p, T = self.nc, self.pools, self.T
        t_b = T - 1 - t_f
        ha = self.hist_all
        hb0 = self.hb_off
        hprev = {"f": ha[:, t_f * 16:(t_f + 1) * 16],
                 "b": ha[:, hb0 + (t_b + 1) * 16:hb0 + (t_b + 2) * 16]}
        tin = t_f % 64
        wxt = self.wx[t_f // 64]
        # gates psum: f at cols 0:64, b at 64:128; one wx load matmul.
        # wx col layout (tin, j, b): slot reads contiguous 64-col windows.
        g_ps = p["g_ps"].tile([128, 128], F32, name="g_ps")
        xstep = 4096 + (63 - tin) * 64 - tin * 64
        nc.tensor.matmul(
            out=g_ps[:],
            lhsT=self.ident[:],
            rhs=A(wxt, tin * 64, [[1, 128], [xstep, 2], [1, 64]]),
            start=True, stop=False,
        )
        # two 1x1 dummy matmuls sharing the h dependency: absorb the PE
        # pipeline-fill / low-p-state cost so the real whh matmuls run warm
        ha1 = self.hist_all[0:1, t_f * 16:t_f * 16 + 1]
        nc.tensor.matmul(out=self.scr_ps[0:1, 0:1], lhsT=self.ident[0:1, 0:1],
                         rhs=ha1, start=True, stop=True)
        nc.tensor.matmul(out=self.scr_ps[0:1, 1:2], lhsT=self.ident[0:1, 0:1],
                         rhs=ha1, start=True, stop=True)
        for di, d in enumerate(("f", "b")):
            for j in range(4):
                nc.tensor.matmul(
                    out=g_ps[:, di * 64 + j * 16:di * 64 + (j + 1) * 16],
                    lhsT=self.whhT[d][:, j * 128:(j + 1) * 128],
                    rhs=hprev[d], start=False, stop=(di == 1 and j == 3),
                )
        act = p["act"].tile([128, 128], F32, name="act")
        nc.scalar.activation(
            out=A(act, 0, [[1, 128], [64, 2], [1, 48]]),
            in_=A(g_ps, 0, [[1, 128], [64, 2], [1, 48]]), func=AF.Sigmoid)
        c = self.cboth
        m2 = p["m2"].tile([128, 32], F32, name="m2")
        nc.vector.tensor_mul(
            out=m2[:],
            in0=A(act, 16, [[1, 128], [64, 2], [1, 16]]),
            in1=c[:],
        )
        nc.scalar.activation(
            out=A(act, 48, [[1, 128], [64, 2], [1, 16]]),
            in_=A(g_ps, 48, [[1, 128], [64, 2], [1, 16]]), func=AF.Tanh)
        m1 = p["m1"].tile([128, 32], F32, name="m1")
        nc.vector.tensor_mul(
            out=m1[:],
            in0=A(act, 0, [[1, 128], [64, 2], [1, 16]]),
            in1=A(act, 48, [[1, 128], [64, 2], [1, 16]]),
        )
        nc.vector.tensor_add(out=c[:], in0=m1[:], in1=m2[:])
        # tiny PE warm-up matmul, data-dependent on m1 so it fires shortly
        # before the next whh burst and lifts the PE out of its low p-state
        tc_ = p["tc2"].tile([128, 32], F32, name="tc2")
        nc.scalar.activation(out=tc_[:], in_=c[:], func=AF.Tanh)
        # merged h write: f dest at (t_f+1)*16, b dest at hb0 + t_b*16
        hstep = hb0 + t_b * 16 - (t_f + 1) * 16
        nc.vector.tensor_mul(
            out=A(ha, (t_f + 1) * 16, [[1, 128], [hstep, 2], [1, 16]]),
            in0=A(act, 32, [[1, 128], [64, 2], [1, 16]]),
            in1=A(tc_, 0, [[1, 128], [16, 2], [1, 16]]),
        )

    # ---- full pipelined emission ---------------------------------------
    def emit_recurrence(self):
        T, NBLK = self.T, self.NBLK
        # prologue: first-needed halves of pair 0 only; rest drips into blk 0
        items0 = self.production_items(0)
        for it in items0[:self.PROLOGUE_ITEMS]:
            it()
        leftover = items0[self.PROLOGUE_ITEMS:]
        for blk in range(NBLK):
            todo = leftover
            leftover = []
            if blk + 1 < NBLK:
                todo = todo + self.production_items(blk + 1)
            k = 0
            for tin in range(64):
                self.slot(blk * 64 + tin)
                want = ((tin + 1) * len(todo)) // 64
                while k < want:
                    todo[k]()
                    k += 1

    def emit_scores(self):
        nc, p, T = self.nc, self.pools, self.T
        NS = T * 16 // 128
        sb = p["sc_sb"].tile([128, 4 * NS], F32)
        for n in range(NS):
            ps = p["sc_ps"].tile([128, 4], F32)
            nc.tensor.matmul(out=ps[:], lhsT=self.hist["f"][:, 16 + n * 128: 16 + (n + 1) * 128],
                             rhs=self.wlabT["f"][:], start=True, stop=False)
            nc.tensor.matmul(out=ps[:], lhsT=self.hist["b"][:, n * 128:(n + 1) * 128],
                             rhs=self.wlabT["b"][:], start=False, stop=True)
            nc.vector.tensor_copy(sb[:, n * 4:(n + 1) * 4], ps[:])
        nc.sync.dma_start(
            bass.AP(self.dr["scores"], 0, [[4, 128], [512, NS], [1, 4]]),
            A(sb, 0, [[1, 128], [4, NS], [1, 4]]),
        )


def build_exmask_b(lens, T):
    import numpy as np
    # exmask_b[blk, c]: -60 at invalid (t, b) for the i/f/o gates, 0 for the
    # g gate; col layout c = tin*64 + j*16 + b.
    NBLK = T // 64
    ev = np.zeros((NBLK, 64, 4, 16), np.float32)
    tin = np.arange(64)
    for blk in range(NBLK):
        t = blk * 64 + tin
        inv = -60.0 * (t[:, None] >= lens[None, :])  # [64, 16]
        ev[blk, :, :3, :] = inv[:, None, :]
    return ev.astype(np.float16).reshape(-1)


def lstm_dram_specs(T=512):
    return [
        ("emb16", [8000, 256], F16), ("tokens_tr", [128, T * 16 // 128], I32),
        ("wihT_f0", [128, 512], F16), ("wihT_f1", [128, 512], F16),
        ("wihT_b0", [128, 512], F16), ("wihT_b1", [128, 512], F16),
        ("whhT_f", [128, 512], F16), ("whhT_b", [128, 512], F16),
        ("biasT_f", [128, 4], F32), ("biasT_b", [128, 4], F32),
        ("exmask_b", [8 * 4096], F16),
        ("wlabT_f", [128, 4], F16), ("wlabT_b", [128, 4], F16),
        ("ident", [128, 128], F16), ("ident32", [128, 16], F32),
    ]


def make_pools(ctx_persist, ctx_trans, tc):
    p = {}
    p["hist"] = ctx_persist.enter_context(tc.tile_pool(name="hist", bufs=1))
    p["state"] = ctx_persist.enter_context(tc.tile_pool(name="state", bufs=1))
    p["wts"] = ctx_persist.enter_context(tc.tile_pool(name="wts", bufs=1))
    p["extra"] = ctx_trans.enter_context(tc.tile_pool(name="extra", bufs=2))
    p["xg"] = ctx_trans.enter_context(tc.tile_pool(name="xg", bufs=16))
    p["xt"] = ctx_trans.enter_context(tc.tile_pool(name="xt", bufs=4))
    p["wx"] = ctx_trans.enter_context(tc.tile_pool(name="wx", bufs=2))
    p["tp_ps"] = ctx_trans.enter_context(tc.tile_pool(name="tp_ps", bufs=2, space="PSUM"))
    p["wx_ps"] = ctx_trans.enter_context(tc.tile_pool(name="wx_ps", bufs=2, space="PSUM"))
    p["g_ps"] = ctx_trans.enter_context(tc.tile_pool(name="g_ps", bufs=2, space="PSUM"))
    p["warm_ps"] = ctx_trans.enter_context(tc.tile_pool(name="warm_ps", bufs=1, space="PSUM"))
    for nm in ("act", "m1", "m2", "tc2"):
        p[nm] = ctx_trans.enter_context(tc.tile_pool(name=nm, bufs=2))
    return p


def make_score_pools(ctx, tc):
    p = {}
    p["sc_ps"] = ctx.enter_context(tc.tile_pool(name="sc_ps", bufs=8, space="PSUM"))
    p["sc_sb"] = ctx.enter_context(tc.tile_pool(name="sc_sb", bufs=1))
    return p


# ---------------------------------------------------------------------------
# DRAM declarations + host prep + SPMD driver
# ---------------------------------------------------------------------------

def _build_program():
    nc = bass.Bass(trn_type="TRN2")
    dr = {}
    for name, shape, dt in lstm_dram_specs(T):
        dr[name] = nc.dram_tensor(name, shape, dt, kind="ExternalInput")
    for name, shape, dt in CRF_DRAM_SPECS:
        dr[name] = nc.dram_tensor(name, shape, dt, kind="ExternalInput")
    for name, shape, dt in CRF_SCRATCH_SPECS:
        dr[name] = nc.dram_tensor(name, shape, dt)
    dr["scores"] = nc.dram_tensor("scores", [T * 16, 4], F32)
    dr["labels"] = nc.dram_tensor("labels", [NB, T], I32, kind="ExternalOutput")

    with tile.TileContext(nc) as tc:
        with ExitStack() as ctx:
            with ExitStack() as ctx_trans:
                pools = make_pools(ctx, ctx_trans, tc)
                em = LstmEmitter(nc, tc, dr, T, pools)
                em.emit_recurrence()
            spools = make_score_pools(ctx, tc)
            pools.update(spools)
            em.emit_scores()
            with ExitStack() as ctx_crf:
                crf_pool = ctx_crf.enter_context(tc.tile_pool(name="crf", bufs=1))
                emit_crf(nc, tc, dr, crf_pool)
    return nc


_CACHE = {}
LAST_EXEC_NS = None


def kernel(**inputs):
    global LAST_EXEC_NS
    _apply_patches()
    from concourse.bass_utils import run_bass_kernel_spmd

    inp = {k: np.asarray(v) for k, v in inputs.items()}
    if "nc" not in _CACHE:
        _CACHE["nc"] = _build_program()
    nc = _CACHE["nc"]

    # shared (batch-independent) host arrays
    shared = {}
    shared["emb16"] = inp["emb"].astype(np.float16)
    perm = np.concatenate([np.arange(128), 128 + np.arange(128),
                           384 + np.arange(128), 256 + np.arange(128)])
    for d, sfx in (("f", "_f"), ("b", "_b")):
        wih = inp[f"W_ih{sfx}"][perm].astype(np.float32)
        whh = inp[f"W_hh{sfx}"][perm].astype(np.float32)
        bias = (inp[f"b_ih{sfx}"] + inp[f"b_hh{sfx}"])[perm].astype(np.float32)
        shared[f"wihT_{d}0"] = np.ascontiguousarray(wih.T[:128]).astype(np.float16)
        shared[f"wihT_{d}1"] = np.ascontiguousarray(wih.T[128:]).astype(np.float16)
        shared[f"whhT_{d}"] = np.ascontiguousarray(whh.T).astype(np.float16)
        shared[f"biasT_{d}"] = np.ascontiguousarray(
            bias.reshape(4, 128).T).astype(np.float32)

        wl = inp["W_lab"].astype(np.float32)
        half = wl[:, :128] if d == "f" else wl[:, 128:]
        shared[f"wlabT_{d}"] = np.ascontiguousarray(half.T).astype(np.float16)
    shared["ident"] = np.eye(128, dtype=np.float16)
    shared["ident32"] = np.eye(128, 16, dtype=np.float32)

    trans = inp["transitions"].astype(np.float32)
    fromB = inp["from_BOS"].astype(np.float32)
    toEOS = inp["to_EOS"].astype(np.float32)
    b_lab = inp["b_lab"].astype(np.float32)

    pad_seq = inp["pad_seq"].astype(np.int64)
    lens_full = inp["lens"].astype(np.int64)

    in_maps = []
    for core in range(NCORES):
        b0 = core * NB
        seq = pad_seq[b0:b0 + NB]
        lens = lens_full[b0:b0 + NB]
        m = dict(shared)
        tok = np.ascontiguousarray(seq.T).reshape(-1).astype(np.int32)
        m["tokens_tr"] = np.ascontiguousarray(tok.reshape(T * 16 // 128, 128).T)
        m["exmask_b"] = build_exmask_b(lens, T)
        m.update(host_crf_consts(lens, trans, fromB, toEOS, b_lab))
        in_maps.append(m)

    res = run_bass_kernel_spmd(nc, in_maps, list(range(NCORES)))
    LAST_EXEC_NS = res.exec_time_ns
    out = np.concatenate([res.results[c]["labels"] for c in range(NCORES)], axis=0)
    return out.astype(np.int32)



# revision 63
# speedup vs baseline: 1.1042x; 1.0569x over previous
"""BiLSTM-CRF Trainium2 kernel (Bass/Tile), data-parallel over batch on 8
NeuronCores. Self-contained: host prep + device emission + SPMD runner.

Pipeline per core (16 sequences, T=512):
  embedding gather (indirect DMA, fp16) -> PE transpose -> Wx matmuls (fp16),
  software-pipelined with the serial BiLSTM recurrence (gates on partitions,
  fp16 weights/hist, fp32 cell state) -> emission scores matmul -> blocked
  Viterbi forward scan + blocked backtrace (max-plus / one-hot map composition
  in 32 chunks of 16 steps, vectorized across 128 partitions).
"""
import sys
import types
import numpy as np

import concourse.bass as bass
import concourse.mybir as mybir
from concourse import tile
from concourse.vector_clock import ScopedClock
import bass_rust
from contextlib import ExitStack

F16 = mybir.dt.float16
F32 = mybir.dt.float32
I32 = mybir.dt.int32
AF = mybir.ActivationFunctionType
AX = mybir.AxisListType.X
OP = mybir.AluOpType

B_FULL, T, V, D = 128, 512, 8000, 256
NB = 16          # sequences per core
NCORES = 8


# ---------------------------------------------------------------------------
# Harness workarounds: walrus in this environment accepts only ONE sync-wait
# per instruction; split extras onto NoOps (BIR json pass) and chunk the Tile
# exit drain. Also register the NTFF profile hook shim so BASS_TRACE=1 works.
# ---------------------------------------------------------------------------
import json as _json

_SW_CTR = [0]


def _split_sync_waits(bir_json: bytes) -> bytes:
    d = _json.loads(bir_json)
    changed = False
    for fn in d.get("functions", []):
        for blk in fn.get("blocks", []):
            new_insts = []
            for inst in blk.get("instructions", []):
                si = inst.get("sync_info")
                waits = (si or {}).get("on_wait") or []
                if len(waits) > 1:
                    changed = True
                    for w in waits[:-1]:
                        _SW_CTR[0] += 1
                        nop = {
                            "engine": inst["engine"],
                            "ins": [],
                            "outs": [],
                            "name": f"I-swsplit-{_SW_CTR[0]}",
                            "opcode": "NoOp",
                            "sync_info": {"on_update": [], "on_wait": [w]},
                        }
                        if "debug" in inst:
                            nop["debug"] = inst["debug"]
                        new_insts.append(nop)
                    si["on_wait"] = [waits[-1]]
                new_insts.append(inst)
            blk["instructions"] = new_insts
    return _json.dumps(d).encode() if changed else bir_json


def _patched_drain_and_barrier(self, tick_clock, wait_clock):
    drain_inst = self.nc.sync.drain()
    wait_clock.add_sem_waits(
        drain_inst.ins, ScopedClock({None: tick_clock.global_clock})
    )
    si = drain_inst.ins.sync_info
    if si is not None and si.on_wait is not None and len(si.on_wait) > 1:
        waits = list(si.on_wait)
        drain_inst.ins.sync_info = bass_rust.SyncInfo(
            on_wait=waits[:1], on_update=list(si.on_update or [])
        )
        for i in range(1, len(waits)):
            nop = self.nc.sync.nop()
            nop.ins.sync_info = bass_rust.SyncInfo(on_wait=[waits[i]], on_update=[])
    self.nc.all_engine_barrier()
    assert self.sems is not None
    popped = self.nc._tile_sem_poison_stack.pop()
    assert popped is self._sem_poison
    self.nc.clear_and_free_semaphores(list(self.sems.allocated().values()))
    self.nc.all_engine_barrier()


_PATCHED = [False]


def _apply_patches():
    if _PATCHED[0]:
        return
    _PATCHED[0] = True
    tile.TileContext._drain_and_barrier = _patched_drain_and_barrier
    import concourse.bass_utils as _bu
    import concourse.bass2jax as _b2j

    _orig_compile = _bu.compile_bir_kernel

    def _wrapped(bir_json, tmpdir, neff_name="file.neff"):
        return _orig_compile(_split_sync_waits(bir_json), tmpdir, neff_name)

    _wrapped._swsplit_wrapped = True
    _bu.compile_bir_kernel = _wrapped
    _b2j.compile_bir_kernel = _wrapped

    if "antenv.axon_hooks" not in sys.modules:
        try:
            import trn_agent_boot.trn_boot as _tb
            _hook = _tb._ntff_profile_via_ctypes("/opt/axon/libaxon_pjrt.so")
        except Exception:
            _hook = None
        m = types.ModuleType("antenv.axon_hooks")
        m.get_axon_ntff_profile_hook = lambda: _hook
        m.set_axon_ntff_profile_hook = lambda h: None
        sys.modules["antenv.axon_hooks"] = m





def A(t, off, dims, p0=0):
    # t: pool tile AP [[rowsize, P], [1, rowsize]]. dims[0] is the partition
    # pair whose step is replaced by the tile's canonical per-partition row
    # size; off is the within-partition element offset.
    rs = t.ap[0][0]
    d = [list(x) for x in dims]
    d[0] = [rs, d[0][1]]
    return bass.AP(t.tensor, t.offset + p0 * rs + off, d)


def AD(handle, off, dims):
    return bass.AP(handle, off, [list(d) for d in dims])


def emit_crf(nc, tc, dr, pool):
    """dr: dict of DRAM handles. pool: sbuf tile pool to allocate from."""
    v = nc.vector

    # ---- V0: build T matrices ------------------------------------------
    scT = pool.tile([128, 256], F32)   # (chpos, s, c)
    # scores_dram is tok-major [8192, 4]: addr = (t*16+b)*4 + c
    for g in range(8):
        nc.sync.dma_start(
            A(scT, 0, [[1, 16], [4, 64], [1, 4]], p0=g * 16),
            AD(dr["scores"], g * 4096, [[4, 16], [64, 64], [1, 4]]),
        )
    transb_sb = pool.tile([128, 16], F32)
    nc.sync.dma_start(transb_sb[:], dr["transb16"][None, :].to_broadcast((128, 16)))
    imp_sb = pool.tile([128, 16], F32)
    nc.sync.dma_start(imp_sb[:], dr["impflat"][None, :].to_broadcast((128, 16)))
    vmask_sb = pool.tile([128, 1024], I32)
    nc.sync.dma_start(vmask_sb[:], dr["vmask128"][:])
    fromBp_sb = pool.tile([128, 4], F32)
    nc.sync.dma_start(fromBp_sb[:], dr["fromBp4"][None, :].to_broadcast((128, 4)))

    Traw = pool.tile([128, 1024], F32)  # (chpos, s, p, c)
    v.tensor_add(
        out=A(Traw, 0, [[1, 128], [256, 4], [16, 16], [4, 4], [1, 4]]),
        in0=A(scT, 0, [[1, 128], [64, 4], [4, 16], [0, 4], [1, 4]]),
        in1=A(transb_sb, 0, [[1, 128], [0, 4], [0, 16], [4, 4], [1, 4]]),
    )
    T128 = pool.tile([128, 1024], F32)
    v.select(
        out=A(T128, 0, [[1, 128], [256, 4], [16, 16], [4, 4], [1, 4]]),
        mask=A(vmask_sb, 0, [[1, 128], [256, 4], [16, 16], [4, 4], [1, 4]]),
        on_true=A(Traw, 0, [[1, 128], [256, 4], [16, 16], [4, 4], [1, 4]]),
        on_false=A(imp_sb, 0, [[1, 128], [0, 4], [0, 16], [4, 4], [1, 4]]),
    )
    # step 0 (partitions 0:16, chpos=0, s=0): T = e0 + fromBp (rows equal)
    v.tensor_add(
        out=A(T128, 0, [[1, 16], [4, 4], [1, 4]]),
        in0=A(scT, 0, [[1, 16], [0, 4], [1, 4]]),
        in1=A(fromBp_sb, 0, [[1, 16], [0, 4], [1, 4]]),
    )

    # ---- V1: chunk max-plus products -----------------------------------
    Ma = pool.tile([128, 64], F32)   # (chpos, i, k/j)
    Mb = pool.tile([128, 64], F32)
    tmp256 = pool.tile([128, 1024], F32)
    v.tensor_copy(
        A(Ma, 0, [[1, 128], [16, 4], [4, 4], [1, 4]]),
        A(T128, 0, [[1, 128], [256, 4], [4, 4], [1, 4]]),
    )
    cur, nxt = Ma, Mb
    for s in range(1, 16):
        v.tensor_add(
            out=A(tmp256, 0, [[1, 128], [64, 4], [16, 4], [4, 4], [1, 4]]),
            in0=A(cur, 0, [[1, 128], [16, 4], [4, 4], [1, 4], [0, 4]]),
            in1=A(T128, s * 16, [[1, 128], [256, 4], [0, 4], [4, 4], [1, 4]]),
        )
        v.tensor_reduce(
            out=A(nxt, 0, [[1, 128], [16, 4], [4, 4], [1, 4]]),
            in_=A(tmp256, 0, [[1, 128], [64, 4], [16, 4], [1, 4], [4, 4]]),
            axis=AX, op=OP.max,
        )
        cur, nxt = nxt, cur
    nc.sync.dma_start(
        AD(dr["mdram"], 0, [[64, 128], [1, 64]]),
        A(cur, 0, [[1, 128], [1, 64]]),
    )

    # ---- V2: serial chunk scan (16 partitions) -------------------------
    M16 = pool.tile([16, 512], F32)
    for g in range(8):
        nc.sync.dma_start(
            A(M16, g * 64, [[1, 16], [1, 64]]),
            AD(dr["mdram"], g * 1024, [[64, 16], [1, 64]]),
        )
    Ball = pool.tile([16, 132], F32)
    v.memset(Ball[:], 0.0)
    t16 = pool.tile([16, 16], F32)
    for c in range(32):
        v.tensor_add(
            out=A(t16, 0, [[1, 16], [4, 4], [1, 4]]),
            in0=A(Ball, c * 4, [[1, 16], [1, 4], [0, 4]]),
            in1=A(M16, c * 16, [[1, 16], [4, 4], [1, 4]]),
        )
        v.tensor_reduce(
            out=A(Ball, (c + 1) * 4, [[1, 16], [1, 4]]),
            in_=A(t16, 0, [[1, 16], [1, 4], [4, 4]]),
            axis=AX, op=OP.max,
        )
    # last label one-hot
    toEOS_sb = pool.tile([16, 4], F32)
    nc.sync.dma_start(toEOS_sb[:], dr["toEOS4"][None, :].to_broadcast((16, 4)))
    c3lab_sb = pool.tile([16, 4], F32)
    nc.sync.dma_start(c3lab_sb[:], dr["c3lab4"][None, :].to_broadcast((16, 4)))
    wiota16 = pool.tile([16, 4], F32)
    nc.sync.dma_start(wiota16[:], dr["wiota4"][None, :].to_broadcast((16, 4)))
    fin = pool.tile([16, 4], F32)
    v.tensor_add(out=fin[:], in0=A(Ball, 128, [[1, 16], [1, 4]]), in1=toEOS_sb[:])
    lmax = pool.tile([16, 1], F32)
    v.tensor_reduce(out=lmax[:], in_=fin[:], axis=AX, op=OP.max)
    loh = pool.tile([16, 4], F32)
    v.tensor_tensor(out=loh[:], in0=fin[:],
                    in1=A(lmax, 0, [[1, 16], [0, 4]]), op=OP.is_equal)
    lohm = pool.tile([16, 4], F32)
    v.tensor_mul(out=lohm[:], in0=loh[:], in1=c3lab_sb[:])
    lenc = pool.tile([16, 1], F32)
    v.tensor_reduce(out=lenc[:], in_=lohm[:], axis=AX, op=OP.max)
    llval = pool.tile([16, 1], F32)
    v.tensor_scalar(out=llval[:], in0=lenc[:], scalar1=-1.0, scalar2=3.0,
                    op0=OP.mult, op1=OP.add)
    lloh = pool.tile([16, 4], F32)
    v.tensor_tensor(out=lloh[:], in0=wiota16[:],
                    in1=A(llval, 0, [[1, 16], [0, 4]]), op=OP.is_equal)
    nc.sync.dma_start(AD(dr["lldram"], 0, [[4, 16], [1, 4]]), lloh[:])
    nc.sync.dma_start(AD(dr["edram"], 0, [[132, 16], [1, 132]]), Ball[:])

    # ---- V3: replay -> backtrace tables --------------------------------
    c3p_sb = pool.tile([128, 16], F32)
    nc.sync.dma_start(c3p_sb[:], dr["c3p16"][None, :].to_broadcast((128, 16)))
    bestA = pool.tile([128, 16], F32)
    bestB = pool.tile([128, 16], F32)
    nc.sync.dma_start(
        bestA[:], AD(dr["edram"], 0, [[16, 8], [132, 16], [4, 4], [1, 4]])
    )
    BT = pool.tile([128, 256], F32)     # (chpos, s, c)
    smat = pool.tile([128, 64], F32)
    oh64 = pool.tile([128, 64], F32)
    enc128 = pool.tile([128, 16], F32)
    bcur, bnxt = bestA, bestB
    for s in range(16):
        v.tensor_add(
            out=A(smat, 0, [[1, 128], [16, 4], [4, 4], [1, 4]]),
            in0=A(bcur, 0, [[1, 128], [4, 4], [1, 4], [0, 4]]),
            in1=A(T128, s * 16, [[1, 128], [256, 4], [4, 4], [1, 4]]),
        )
        v.tensor_reduce(
            out=A(bnxt, 0, [[1, 128], [4, 4], [1, 4]]),
            in_=A(smat, 0, [[1, 128], [16, 4], [1, 4], [4, 4]]),
            axis=AX, op=OP.max,
        )
        v.tensor_tensor(
            out=A(oh64, 0, [[1, 128], [16, 4], [4, 4], [1, 4]]),
            in0=A(smat, 0, [[1, 128], [16, 4], [4, 4], [1, 4]]),
            in1=A(bnxt, 0, [[1, 128], [4, 4], [0, 4], [1, 4]]),
            op=OP.is_equal,
        )
        v.tensor_mul(
            out=A(oh64, 0, [[1, 128], [16, 4], [4, 4], [1, 4]]),
            in0=A(oh64, 0, [[1, 128], [16, 4], [4, 4], [1, 4]]),
            in1=A(c3p_sb, 0, [[1, 128], [0, 4], [4, 4], [1, 4]]),
        )
        v.tensor_reduce(
            out=A(enc128, 0, [[1, 128], [4, 4], [1, 4]]),
            in_=A(oh64, 0, [[1, 128], [16, 4], [1, 4], [4, 4]]),
            axis=AX, op=OP.max,
        )
        v.tensor_scalar(
            out=A(BT, s * 4, [[1, 128], [64, 4], [1, 4]]),
            in0=A(enc128, 0, [[1, 128], [4, 4], [1, 4]]),
            scalar1=-1.0, scalar2=3.0, op0=OP.mult, op1=OP.add,
        )
        bcur, bnxt = bnxt, bcur
    nc.sync.dma_start(
        AD(dr["btdram"], 0, [[256, 128], [1, 256]]),
        A(BT, 0, [[1, 128], [1, 256]]),
    )

    # ---- VA: backtrace map tables + chunk compositions -----------------
    BTS = pool.tile([128, 256], F32)
    # top group's last slot is never used; zero-fill before partial overwrite
    v.memset(A(BTS, 252, [[1, 128], [1, 4]]), 0.0)
    nc.sync.dma_start(
        A(BTS, 0, [[1, 128], [1, 252]]),
        AD(dr["btdram"], 4, [[256, 128], [1, 252]]),
    )
    # last slot of each partition: first bt entry of the next chunk group
    nc.sync.dma_start(
        A(BTS, 252, [[1, 112], [1, 4]]),
        AD(dr["btdram"], 16 * 256, [[256, 112], [1, 4]]),
    )
    meq_sb = pool.tile([128, 64], I32)
    mlt_sb = pool.tile([128, 64], I32)
    nc.sync.dma_start(meq_sb[:], dr["meq128"][:])
    nc.sync.dma_start(mlt_sb[:], dr["mlt128"][:])
    lloh128 = pool.tile([128, 4], F32)
    nc.sync.dma_start(lloh128[:], AD(dr["lldram"], 0, [[0, 8], [4, 16], [1, 4]]))
    i4_sb = pool.tile([128, 16], F32)
    nc.sync.dma_start(i4_sb[:], dr["i4flat"][None, :].to_broadcast((128, 16)))
    wiota128 = pool.tile([128, 4], F32)
    nc.sync.dma_start(wiota128[:], dr["wiota4"][None, :].to_broadcast((128, 4)))

    Fall = pool.tile([128, 1024], F32)  # (chpos, s, u, w)
    tmpA = pool.tile([128, 64], F32)
    for s in range(16):
        # oh(u,w) = bt_{t+1}[u] == w
        v.tensor_tensor(
            out=A(tmpA, 0, [[1, 128], [16, 4], [4, 4], [1, 4]]),
            in0=A(BTS, s * 4, [[1, 128], [64, 4], [1, 4], [0, 4]]),
            in1=A(wiota128, 0, [[1, 128], [0, 4], [0, 4], [1, 4]]),
            op=OP.is_equal,
        )
        # tmp2 = meq ? lloh : I4  ; F = mlt ? oh : tmp2  (write into Fall)
        v.select(
            out=A(Fall, s * 16, [[1, 128], [256, 4], [4, 4], [1, 4]]),
            mask=A(meq_sb, s, [[1, 128], [16, 4], [0, 4], [0, 4]]),
            on_true=A(lloh128, 0, [[1, 128], [0, 4], [0, 4], [1, 4]]),
            on_false=A(i4_sb, 0, [[1, 128], [0, 4], [4, 4], [1, 4]]),
        )
        v.select(
            out=A(Fall, s * 16, [[1, 128], [256, 4], [4, 4], [1, 4]]),
            mask=A(mlt_sb, s, [[1, 128], [16, 4], [0, 4], [0, 4]]),
            on_true=A(tmpA, 0, [[1, 128], [16, 4], [4, 4], [1, 4]]),
            on_false=A(Fall, s * 16, [[1, 128], [256, 4], [4, 4], [1, 4]]),
        )
    Ga = pool.tile([128, 64], F32)
    Gb = pool.tile([128, 64], F32)
    v.tensor_copy(
        A(Ga, 0, [[1, 128], [16, 4], [4, 4], [1, 4]]),
        A(Fall, 15 * 16, [[1, 128], [256, 4], [4, 4], [1, 4]]),
    )
    gcur, gnxt = Ga, Gb
    for s in range(14, -1, -1):
        v.tensor_mul(
            out=A(tmp256, 0, [[1, 128], [64, 4], [16, 4], [4, 4], [1, 4]]),
            in0=A(gcur, 0, [[1, 128], [16, 4], [4, 4], [1, 4], [0, 4]]),
            in1=A(Fall, s * 16, [[1, 128], [256, 4], [0, 4], [4, 4], [1, 4]]),
        )
        v.tensor_reduce(
            out=A(gnxt, 0, [[1, 128], [16, 4], [4, 4], [1, 4]]),
            in_=A(tmp256, 0, [[1, 128], [64, 4], [16, 4], [1, 4], [4, 4]]),
            axis=AX, op=OP.max,
        )
        gcur, gnxt = gnxt, gcur
    nc.sync.dma_start(
        AD(dr["gdram"], 0, [[64, 128], [1, 64]]),
        A(gcur, 0, [[1, 128], [1, 64]]),
    )

    # ---- VB: serial reverse chunk scan (16 partitions) -----------------
    Gall16 = pool.tile([16, 512], F32)
    for g in range(8):
        nc.sync.dma_start(
            A(Gall16, g * 64, [[1, 16], [1, 64]]),
            AD(dr["gdram"], g * 1024, [[64, 16], [1, 64]]),
        )
    EB = pool.tile([16, 132], F32)
    nc.sync.dma_start(
        A(EB, 128, [[1, 16], [1, 4]]), dr["e0oh4"][None, :].to_broadcast((16, 4))
    )
    tb16 = pool.tile([16, 16], F32)
    for c in range(31, -1, -1):
        v.tensor_mul(
            out=tb16[:],
            in0=A(EB, (c + 1) * 4, [[1, 16], [1, 4], [0, 4]]),
            in1=A(Gall16, c * 16, [[1, 16], [4, 4], [1, 4]]),
        )
        v.tensor_reduce(
            out=A(EB, c * 4, [[1, 16], [1, 4]]),
            in_=A(tb16, 0, [[1, 16], [1, 4], [4, 4]]),
            axis=AX, op=OP.max,
        )
    nc.sync.dma_start(AD(dr["ebdram"], 0, [[132, 16], [1, 132]]), EB[:])

    # ---- VC: labels -----------------------------------------------------
    cohE = pool.tile([128, 16], F32)
    nc.sync.dma_start(
        cohE[:], AD(dr["ebdram"], 4, [[16, 8], [132, 16], [4, 4], [1, 4]])
    )
    LABOH = pool.tile([128, 256], F32)  # (chpos, s, w)
    tmpc = pool.tile([128, 64], F32)
    for s in range(15, -1, -1):
        if s == 15:
            in0 = A(cohE, 0, [[1, 128], [4, 4], [1, 4], [0, 4]])
        else:
            in0 = A(LABOH, (s + 1) * 4, [[1, 128], [64, 4], [1, 4], [0, 4]])
        v.tensor_mul(
            out=A(tmpc, 0, [[1, 128], [16, 4], [4, 4], [1, 4]]),
            in0=in0,
            in1=A(Fall, s * 16, [[1, 128], [256, 4], [4, 4], [1, 4]]),
        )
        v.tensor_reduce(
            out=A(LABOH, s * 4, [[1, 128], [64, 4], [1, 4]]),
            in_=A(tmpc, 0, [[1, 128], [16, 4], [1, 4], [4, 4]]),
            axis=AX, op=OP.max,
        )
    omask_sb = pool.tile([128, 64], F32)
    nc.sync.dma_start(omask_sb[:], dr["outmask128"][:])
    labv = pool.tile([128, 64], F32)
    tmpl = pool.tile([128, 256], F32)
    v.tensor_mul(
        out=A(tmpl, 0, [[1, 128], [64, 4], [4, 16], [1, 4]]),
        in0=A(LABOH, 0, [[1, 128], [64, 4], [4, 16], [1, 4]]),
        in1=A(wiota128, 0, [[1, 128], [0, 4], [0, 16], [1, 4]]),
    )
    v.tensor_reduce(
        out=A(labv, 0, [[1, 128], [16, 4], [1, 16]]),
        in_=A(tmpl, 0, [[1, 128], [64, 4], [4, 16], [1, 4]]),
        axis=AX, op=OP.add,
    )
    v.tensor_mul(out=labv[:], in0=labv[:], in1=omask_sb[:])
    labi = pool.tile([128, 64], I32)
    v.tensor_copy(labi[:], labv[:])
    for cp in range(4):
        nc.sync.dma_start(
            AD(dr["labels"], 16 * cp, [[64, 8], [512, 16], [1, 16]]),
            A(labi, cp * 16, [[1, 128], [1, 16]]),
        )


def host_crf_consts(lens, trans, fromB, toEOS, b_lab):
    """All host-side constant arrays, keyed to match dram handle names."""
    import numpy as np
    T, B, L = 512, 16, 4
    NEG = -1e9
    chgrp = np.arange(8)
    out = {}
    out["transb16"] = (trans + b_lab[None, :]).astype(np.float32).reshape(16)
    imp = np.full((L, L), NEG, np.float32)
    np.fill_diagonal(imp, 0.0)
    out["impflat"] = imp.reshape(16)
    out["fromBp4"] = (fromB + b_lab).astype(np.float32)
    out["toEOS4"] = toEOS.astype(np.float32)
    out["c3lab4"] = (3.0 - np.arange(4)).astype(np.float32)
    out["wiota4"] = np.arange(4).astype(np.float32)
    out["c3p16"] = np.repeat(3.0 - np.arange(4), 4).astype(np.float32)
    out["i4flat"] = np.eye(4, dtype=np.float32).reshape(16)
    out["e0oh4"] = np.array([1, 0, 0, 0], np.float32)
    # t value at (P, chpos, s):  P = chgrp*16 + b ; t = 16*(4*chgrp+chpos)+s
    P_chgrp = np.arange(128) // 16
    P_b = np.arange(128) % 16
    chpos = np.arange(4)
    s = np.arange(16)
    tt = 16 * (4 * P_chgrp[:, None, None] + chpos[None, :, None]) + s[None, None, :]
    lb = lens[P_b][:, None, None]
    vm = (tt < lb)
    out["vmask128"] = np.repeat(
        vm.reshape(128, 64)[:, :, None], 16, axis=2
    ).reshape(128, 1024).astype(np.int32)
    out["meq128"] = (tt == lb - 1).reshape(128, 64).astype(np.int32)
    out["mlt128"] = (tt < lb - 1).reshape(128, 64).astype(np.int32)
    out["outmask128"] = (tt < lb).reshape(128, 64).astype(np.float32)
    return out


CRF_DRAM_SPECS = [
    ("transb16", [16], F32), ("impflat", [16], F32), ("fromBp4", [4], F32),
    ("toEOS4", [4], F32), ("c3lab4", [4], F32), ("wiota4", [4], F32),
    ("c3p16", [16], F32), ("i4flat", [16], F32), ("e0oh4", [4], F32),
    ("vmask128", [128, 1024], I32), ("meq128", [128, 64], I32),
    ("mlt128", [128, 64], I32), ("outmask128", [128, 64], F32),
]
CRF_SCRATCH_SPECS = [
    ("mdram", [8192], F32), ("edram", [2112], F32), ("btdram", [32832], F32),
    ("gdram", [8192], F32), ("lldram", [64], F32), ("ebdram", [2112], F32),
]






class LstmEmitter:
    def __init__(self, nc, tc, dr, T, pools):
        self.nc, self.tc, self.dr, self.T = nc, tc, dr, T
        self.NBLK = T // 64
        p = pools
        hist_all = p["hist"].tile([128, 2 * (T + 1) * 16], F16, name="hist_all")
        self.hist_all = hist_all
        self.hb_off = (T + 1) * 16
        self.hist = {
            "f": hist_all[:, 0:(T + 1) * 16],
            "b": hist_all[:, (T + 1) * 16:2 * (T + 1) * 16],
        }
        nc.vector.memset(self.hist["f"][:, 0:16], 0.0)
        nc.vector.memset(self.hist["b"][:, T * 16:(T + 1) * 16], 0.0)
        self.cboth = p["state"].tile([128, 32], F32, name="cboth")
        nc.vector.memset(self.cboth[:], 0.0)
        # weights
        self.whhT = {}
        self.wihT = {}
        self.biasT = {}
        self.wlabT = {}
        for d in ("f", "b"):
            w = p["wts"].tile([128, 512], F16, name=f"whh_{d}")
            nc.sync.dma_start(w[:], dr[f"whhT_{d}"][:])
            self.whhT[d] = w
            hs = []
            for h in range(2):
                wh = p["wts"].tile([128, 512], F16, name=f"wih_{d}{h}")
                nc.sync.dma_start(wh[:], dr[f"wihT_{d}{h}"][:])
                hs.append(wh)
            self.wihT[d] = hs
            bt = p["wts"].tile([128, 4], F32, name=f"bias_{d}")
            nc.sync.dma_start(bt[:], dr[f"biasT_{d}"][:])
            self.biasT[d] = bt
            wl = p["wts"].tile([128, 4], F16, name=f"wlab_{d}")
            nc.sync.dma_start(wl[:], dr[f"wlabT_{d}"][:])
            self.wlabT[d] = wl
        self.ident = p["wts"].tile([128, 128], F16)
        nc.sync.dma_start(self.ident[:], dr["ident"][:])
        self.ident32 = p["wts"].tile([128, 16], F32, name="ident32")
        nc.sync.dma_start(self.ident32[:], dr["ident32"][:])
        self.scr_ps = p["warm_ps"].tile([128, 16], F32, name="scr_ps")
        # all token indices, partition-major: tok_sb[p, c] = token[c*128+p]
        self.tok_sb = p["wts"].tile([128, 64], I32, name="tok_sb")
        nc.sync.dma_start(self.tok_sb[:], dr["tokens_tr"][:])

        self.pools = p
        self.wx = {}   # pair -> tile [128, 8192] fp16 (f blk at 0, b blk at 4096)

    # ---- production of one pair's wx: (f, pair) and (b, NBLK-1-pair) ----
    def production_items(self, pair):
        """Returns a list of closures emitting the gather/transpose/matmul/
        bias work that materializes wx[pair] (both dirs)."""
        nc, dr, p = self.nc, self.dr, self.pools
        items = []
        xg = {}
        state = {}
        blk_of = {"f": pair, "b": self.NBLK - 1 - pair}

        def alloc():
            state["wx"] = p["wx"].tile([128, 8192], F16, name="wx")
            self.wx[pair] = state["wx"]
            state["xt_f"] = p["xt"].tile([128, 2048], F16, name="xt_f")
            state["xt_b"] = p["xt"].tile([128, 2048], F16, name="xt_b")
            # pad mask for the bwd dir, broadcast to all partitions:
            # -60 at invalid (t, b) in the i/f/o chunks, 0 in the g chunk
            exm = p["extra"].tile([128, 4096], F16, name="exmask")
            nc.sync.dma_start(
                exm[:],
                bass.AP(dr["exmask_b"], blk_of["b"] * 4096,
                        [[0, 128], [1, 4096]]),
            )
            state["exm"] = exm

        def gather(d, i):
            def go():
                c = blk_of[d] * 8 + i
                t = p["xg"].tile([128, 256], F16)
                nc.gpsimd.indirect_dma_start(
                    out=t[:], out_offset=None, in_=dr["emb16"][:],
                    in_offset=bass.IndirectOffsetOnAxis(
                        ap=self.tok_sb[:, c:c + 1], axis=0),
                )
                xg[(d, i)] = t
            return go

        def transp2(d, i):
            def go():
                ps = p["tp_ps"].tile([128, 256], F16)
                for h in range(2):
                    nc.tensor.transpose(
                        out=ps[:, h * 128:(h + 1) * 128],
                        in_=xg[(d, i)][:, h * 128:(h + 1) * 128],
                        identity=self.ident[:],
                    )
                nc.vector.tensor_copy(
                    A(state[f"xt_{d}"], i * 128, [[1, 128], [1024, 2], [1, 128]]),
                    A(ps, 0, [[1, 128], [128, 2], [1, 128]]),
                )
            return go

        def mm(d, j, n, h):
            def go():
                if h == 0:
                    state[("ps", d, j, n)] = p["wx_ps"].tile(
                        [128, 512], F32, name="wx_ps")
                nc.tensor.matmul(
                    out=state[("ps", d, j, n)][:],
                    lhsT=self.wihT[d][h][:, j * 128:(j + 1) * 128],
                    rhs=state[f"xt_{d}"][
                        :, h * 1024 + n * 512: h * 1024 + (n + 1) * 512],
                    start=(h == 0), stop=(h == 1),
                )
            return go

        def bias(d, j, n):
            def go():
                # wx col layout is (tin, j, b): per-slot reads are contiguous
                # 64-col windows and each copy stays inside one half.
                d0 = 0 if d == "f" else 4096
                dst = A(state["wx"], d0 + n * 2048 + j * 16,
                        [[1, 128], [64, 32], [1, 16]])
                if d == "f":
                    # fwd: psum -> sbuf with per-partition bias, on ACT
                    nc.scalar.activation(
                        out=dst, in_=state[("ps", d, j, n)][:],
                        func=AF.Identity, bias=self.biasT[d][:, j:j + 1])
                else:
                    # bwd: psum + bias + pad mask, on DVE
                    nc.vector.scalar_tensor_tensor(
                        out=dst, in0=state[("ps", d, j, n)][:],
                        scalar=self.biasT[d][:, j:j + 1],
                        in1=A(state["exm"], n * 2048 + j * 16,
                              [[1, 128], [64, 32], [1, 16]]),
                        op0=OP.add, op1=OP.add,
                    )
            return go

        # first-needed halves first: f consumes n=0 at slots 0..31 of its
        # block, b consumes n=1 (tin_b 63..32). Gathers i 0..3 feed n=0,
        # i 4..7 feed n=1.
        items.append(alloc)
        halves = [(("f", range(0, 4), 0), ("b", range(4, 8), 1)),
                  (("f", range(4, 8), 1), ("b", range(0, 4), 0))]
        for half in halves:
            for d, irng, n in half:
                for i in irng:
                    items.append(gather(d, i))
            for d, irng, n in half:
                for i in irng:
                    items.append(transp2(d, i))
            for j in range(4):
                for d, irng, n in half:
                    items.append(mm(d, j, n, 0))
                    items.append(mm(d, j, n, 1))
                    items.append(bias(d, j, n))
        return items
    # items in the first half: alloc + 8 gathers + 8 transposes + 24 mm/copy
    PROLOGUE_ITEMS = 41

    # ---- one recurrence slot: fwd step t_f and bwd step t_b merged ----
    def slot(self, t_f):
        nc, p, T = self.nc, self.pools, self.T
        t_b = T - 1 - t_f
        ha = self.hist_all
        hb0 = self.hb_off
        hprev = {"f": ha[:, t_f * 16:(t_f + 1) * 16],
                 "b": ha[:, hb0 + (t_b + 1) * 16:hb0 + (t_b + 2) * 16]}
        tin = t_f % 64
        wxt = self.wx[t_f // 64]
        # gates psum, sig-gates contiguous: cols (d, j<3, b) in 0:96,
        # tanh-gates (d, b) in 96:128. wx col layout (tin, j, b).
        g_ps = p["g_ps"].tile([128, 128], F32, name="g_ps")
        xstep = 4096 + (63 - tin) * 64 - tin * 64
        nc.tensor.matmul(
            out=A(g_ps, 0, [[1, 128], [48, 2], [1, 48]]),
            lhsT=self.ident[:],
            rhs=A(wxt, tin * 64, [[1, 128], [xstep, 2], [1, 48]]),
            start=True, stop=False,
        )
        nc.tensor.matmul(
            out=A(g_ps, 96, [[1, 128], [16, 2], [1, 16]]),
            lhsT=self.ident[:],
            rhs=A(wxt, tin * 64 + 48, [[1, 128], [xstep, 2], [1, 16]]),
            start=True, stop=False,
        )
        for di, d in enumerate(("f", "b")):
            for j in range(4):
                oc = di * 48 + j * 16 if j < 3 else 96 + di * 16
                nc.tensor.matmul(
                    out=g_ps[:, oc:oc + 16],
                    lhsT=self.whhT[d][:, j * 128:(j + 1) * 128],
                    rhs=hprev[d], start=False, stop=(di == 1 and j == 3),
                )
        act = p["act"].tile([128, 128], F32, name="act")
        nc.scalar.activation(out=act[:, 0:96], in_=g_ps[:, 0:96],
                             func=AF.Sigmoid)
        c = self.cboth
        m2 = p["m2"].tile([128, 32], F32, name="m2")
        nc.vector.tensor_mul(
            out=m2[:],
            in0=A(act, 16, [[1, 128], [48, 2], [1, 16]]),
            in1=c[:],
        )
        nc.scalar.activation(out=act[:, 96:128], in_=g_ps[:, 96:128],
                             func=AF.Tanh)
        m1 = p["m1"].tile([128, 32], F32, name="m1")
        nc.vector.tensor_mul(
            out=m1[:],
            in0=A(act, 0, [[1, 128], [48, 2], [1, 16]]),
            in1=A(act, 96, [[1, 128], [16, 2], [1, 16]]),
        )
        nc.vector.tensor_add(out=c[:], in0=m1[:], in1=m2[:])
        tc_ = p["tc2"].tile([128, 32], F32, name="tc2")
        nc.scalar.activation(out=tc_[:], in_=c[:], func=AF.Tanh)
        # merged h write: f dest at (t_f+1)*16, b dest at hb0 + t_b*16
        hstep = hb0 + t_b * 16 - (t_f + 1) * 16
        nc.vector.tensor_mul(
            out=A(ha, (t_f + 1) * 16, [[1, 128], [hstep, 2], [1, 16]]),
            in0=A(act, 32, [[1, 128], [48, 2], [1, 16]]),
            in1=A(tc_, 0, [[1, 128], [16, 2], [1, 16]]),
        )

    # ---- full pipelined emission ---------------------------------------
    def emit_recurrence(self):
        T, NBLK = self.T, self.NBLK
        # prologue: first-needed halves of pair 0 only; rest drips into blk 0
        items0 = self.production_items(0)
        for it in items0[:self.PROLOGUE_ITEMS]:
            it()
        leftover = items0[self.PROLOGUE_ITEMS:]
        for blk in range(NBLK):
            todo = leftover
            leftover = []
            if blk + 1 < NBLK:
                todo = todo + self.production_items(blk + 1)
            k = 0
            for tin in range(64):
                self.slot(blk * 64 + tin)
                want = ((tin + 1) * len(todo)) // 64
                while k < want:
                    todo[k]()
                    k += 1

    def emit_scores(self):
        nc, p, T = self.nc, self.pools, self.T
        NS = T * 16 // 128
        sb = p["sc_sb"].tile([128, 4 * NS], F32)
        for n in range(NS):
            ps = p["sc_ps"].tile([128, 4], F32)
            nc.tensor.matmul(out=ps[:], lhsT=self.hist["f"][:, 16 + n * 128: 16 + (n + 1) * 128],
                             rhs=self.wlabT["f"][:], start=True, stop=False)
            nc.tensor.matmul(out=ps[:], lhsT=self.hist["b"][:, n * 128:(n + 1) * 128],
                             rhs=self.wlabT["b"][:], start=False, stop=True)
            nc.vector.tensor_copy(sb[:, n * 4:(n + 1) * 4], ps[:])
        # split across 8 DMA queues (one big transposing DMA serializes
        # ~8k tiny descriptors on a single engine)
        for q in range(8):
            nc.sync.dma_start(
                bass.AP(self.dr["scores"], q * 512 * (NS // 8),
                        [[4, 128], [512, NS // 8], [1, 4]]),
                A(sb, q * 4 * (NS // 8), [[1, 128], [4, NS // 8], [1, 4]]),
            )


def build_exmask_b(lens, T):
    import numpy as np
    # exmask_b[blk, c]: -60 at invalid (t, b) for the i/f/o gates, 0 for the
    # g gate; col layout c = tin*64 + j*16 + b.
    NBLK = T // 64
    ev = np.zeros((NBLK, 64, 4, 16), np.float32)
    tin = np.arange(64)
    for blk in range(NBLK):
        t = blk * 64 + tin
        inv = -60.0 * (t[:, None] >= lens[None, :])  # [64, 16]
        ev[blk, :, :3, :] = inv[:, None, :]
    return ev.astype(np.float16).reshape(-1)


def lstm_dram_specs(T=512):
    return [
        ("emb16", [8000, 256], F16), ("tokens_tr", [128, T * 16 // 128], I32),
        ("wihT_f0", [128, 512], F16), ("wihT_f1", [128, 512], F16),
        ("wihT_b0", [128, 512], F16), ("wihT_b1", [128, 512], F16),
        ("whhT_f", [128, 512], F16), ("whhT_b", [128, 512], F16),
        ("biasT_f", [128, 4], F32), ("biasT_b", [128, 4], F32),
        ("exmask_b", [8 * 4096], F16),
        ("wlabT_f", [128, 4], F16), ("wlabT_b", [128, 4], F16),
        ("ident", [128, 128], F16), ("ident32", [128, 16], F32),
    ]


def make_pools(ctx_persist, ctx_trans, tc):
    p = {}
    p["hist"] = ctx_persist.enter_context(tc.tile_pool(name="hist", bufs=1))
    p["state"] = ctx_persist.enter_context(tc.tile_pool(name="state", bufs=1))
    p["wts"] = ctx_persist.enter_context(tc.tile_pool(name="wts", bufs=1))
    p["extra"] = ctx_trans.enter_context(tc.tile_pool(name="extra", bufs=2))
    p["xg"] = ctx_trans.enter_context(tc.tile_pool(name="xg", bufs=16))
    p["xt"] = ctx_trans.enter_context(tc.tile_pool(name="xt", bufs=4))
    p["wx"] = ctx_trans.enter_context(tc.tile_pool(name="wx", bufs=2))
    p["tp_ps"] = ctx_trans.enter_context(tc.tile_pool(name="tp_ps", bufs=2, space="PSUM"))
    p["wx_ps"] = ctx_trans.enter_context(tc.tile_pool(name="wx_ps", bufs=2, space="PSUM"))
    p["g_ps"] = ctx_trans.enter_context(tc.tile_pool(name="g_ps", bufs=2, space="PSUM"))
    p["warm_ps"] = ctx_trans.enter_context(tc.tile_pool(name="warm_ps", bufs=1, space="PSUM"))
    for nm in ("act", "m1", "m2", "tc2"):
        p[nm] = ctx_trans.enter_context(tc.tile_pool(name=nm, bufs=2))
    return p


def make_score_pools(ctx, tc):
    p = {}
    p["sc_ps"] = ctx.enter_context(tc.tile_pool(name="sc_ps", bufs=8, space="PSUM"))
    p["sc_sb"] = ctx.enter_context(tc.tile_pool(name="sc_sb", bufs=1))
    return p


# ---------------------------------------------------------------------------
# DRAM declarations + host prep + SPMD driver
# ---------------------------------------------------------------------------

def _build_program():
    nc = bass.Bass(trn_type="TRN2")
    dr = {}
    for name, shape, dt in lstm_dram_specs(T):
        dr[name] = nc.dram_tensor(name, shape, dt, kind="ExternalInput")
    for name, shape, dt in CRF_DRAM_SPECS:
        dr[name] = nc.dram_tensor(name, shape, dt, kind="ExternalInput")
    for name, shape, dt in CRF_SCRATCH_SPECS:
        dr[name] = nc.dram_tensor(name, shape, dt)
    dr["scores"] = nc.dram_tensor("scores", [T * 16, 4], F32)
    dr["labels"] = nc.dram_tensor("labels", [NB, T], I32, kind="ExternalOutput")

    with tile.TileContext(nc) as tc:
        with ExitStack() as ctx:
            with ExitStack() as ctx_trans:
                pools = make_pools(ctx, ctx_trans, tc)
                em = LstmEmitter(nc, tc, dr, T, pools)
                em.emit_recurrence()
            spools = make_score_pools(ctx, tc)
            pools.update(spools)
            em.emit_scores()
            with ExitStack() as ctx_crf:
                crf_pool = ctx_crf.enter_context(tc.tile_pool(name="crf", bufs=1))
                emit_crf(nc, tc, dr, crf_pool)
    return nc


_CACHE = {}
LAST_EXEC_NS = None


def kernel(**inputs):
    global LAST_EXEC_NS
    _apply_patches()
    from concourse.bass_utils import run_bass_kernel_spmd

    inp = {k: np.asarray(v) for k, v in inputs.items()}
    if "nc" not in _CACHE:
        _CACHE["nc"] = _build_program()
    nc = _CACHE["nc"]

    # shared (batch-independent) host arrays
    shared = {}
    shared["emb16"] = inp["emb"].astype(np.float16)
    perm = np.concatenate([np.arange(128), 128 + np.arange(128),
                           384 + np.arange(128), 256 + np.arange(128)])
    for d, sfx in (("f", "_f"), ("b", "_b")):
        wih = inp[f"W_ih{sfx}"][perm].astype(np.float32)
        whh = inp[f"W_hh{sfx}"][perm].astype(np.float32)
        bias = (inp[f"b_ih{sfx}"] + inp[f"b_hh{sfx}"])[perm].astype(np.float32)
        shared[f"wihT_{d}0"] = np.ascontiguousarray(wih.T[:128]).astype(np.float16)
        shared[f"wihT_{d}1"] = np.ascontiguousarray(wih.T[128:]).astype(np.float16)
        shared[f"whhT_{d}"] = np.ascontiguousarray(whh.T).astype(np.float16)
        shared[f"biasT_{d}"] = np.ascontiguousarray(
            bias.reshape(4, 128).T).astype(np.float32)

        wl = inp["W_lab"].astype(np.float32)
        half = wl[:, :128] if d == "f" else wl[:, 128:]
        shared[f"wlabT_{d}"] = np.ascontiguousarray(half.T).astype(np.float16)
    shared["ident"] = np.eye(128, dtype=np.float16)
    shared["ident32"] = np.eye(128, 16, dtype=np.float32)

    trans = inp["transitions"].astype(np.float32)
    fromB = inp["from_BOS"].astype(np.float32)
    toEOS = inp["to_EOS"].astype(np.float32)
    b_lab = inp["b_lab"].astype(np.float32)

    pad_seq = inp["pad_seq"].astype(np.int64)
    lens_full = inp["lens"].astype(np.int64)

    in_maps = []
    for core in range(NCORES):
        b0 = core * NB
        seq = pad_seq[b0:b0 + NB]
        lens = lens_full[b0:b0 + NB]
        m = dict(shared)
        tok = np.ascontiguousarray(seq.T).reshape(-1).astype(np.int32)
        m["tokens_tr"] = np.ascontiguousarray(tok.reshape(T * 16 // 128, 128).T)
        m["exmask_b"] = build_exmask_b(lens, T)
        m.update(host_crf_consts(lens, trans, fromB, toEOS, b_lab))
        in_maps.append(m)

    res = run_bass_kernel_spmd(nc, in_maps, list(range(NCORES)))
    LAST_EXEC_NS = res.exec_time_ns
    out = np.concatenate([res.results[c]["labels"] for c in range(NCORES)], axis=0)
    return out.astype(np.int32)

